# revision 1
# baseline (speedup 1.0000x reference)
"""DepthLSSTransform Trainium kernel: 3 SPMD launches over 8 NeuronCores.

Launch A: per-camera conv pipeline (dtransform + depthnet + softmax) on
          24-row bands (one 16-row + one 8-row segment per core).
Launch B: bev_pool segment-sum via one-hot matmuls over a host-built
          virtual-window schedule (sorted-by-voxel points).
Launch C: BEV downsample convs, spatially sharded.
Host: geometry/voxel indices, scheduling, gathers, folds (orchestration).
"""
import numpy as np
import ml_dtypes

import concourse.bass as bass
import concourse.tile as tile
from concourse import bacc, mybir
from concourse.bass_utils import run_bass_kernel_spmd

dt = mybir.dt
bf16 = ml_dtypes.bfloat16

# ---- problem constants (hardcoded per contract) ----
B, N = 1, 6
CIN, CIMG, DD = 256, 80, 59
FH, FW, IH, IW = 32, 88, 256, 704
XY0, DXY, NX = -54.0, 0.3, 360
Z0, DZ, NZ = -10.0, 20.0, 1
NPTS = N * DD * FH * FW
NPIX = N * FH * FW
NCORES = 8
QV = 4                      # chunks of 128 points per virtual window

# per-core segments: (camera, h0) for seg A (16 rows) and seg B (8 rows)
SEG_A = [(0, 0), (1, 0), (1, 16), (2, 16), (3, 0), (4, 0), (4, 16), (5, 16)]
SEG_B = [(0, 16), (0, 24), (2, 0), (2, 8), (3, 16), (3, 24), (5, 0), (5, 8)]
# band pixel ranges in global row order (row = n*32 + h)
ROWS_OF_CORE = [[(SEG_A[c][0] * FH + SEG_A[c][1] + r) for r in range(16)] +
                [(SEG_B[c][0] * FH + SEG_B[c][1] + r) for r in range(8)]
                for c in range(NCORES)]

# segment geometry: rows16 segment: d rows [8h0-34, 8h0+158) (192), dt2 out
# rows [2h0-8, 2h0+39) (47), dt3 [h0-3, h0+19) (22), dn1 [h0-1, h0+17) (18)
SEGS = [dict(nout=16, nd=192, nq=48, nt2=47, nt3=22, nn1=18),
        dict(nout=8, nd=128, nq=32, nt2=31, nt3=14, nn1=10)]


def _seg_ranges(h0, S):
    return dict(d0=8 * h0 - 34, q0=2 * h0 - 8, t0=h0 - 3, r0=h0 - 1, o0=h0)


# ---------------------------------------------------------------- launch A
def build_launch_a(debug=False, psum_bufs=3, work_bufs=3, stages=9):
    nc = bacc.Bacc("TRN2", target_bir_lowering=False, debug=False,
                   num_devices=NCORES)
    AP = {}

    def inp(name, shape, dtype=dt.bfloat16):
        AP[name] = nc.dram_tensor(name, shape, dtype, kind="ExternalInput").ap()
        return AP[name]

    # per segment inputs (s = 0: 16-row, 1: 8-row)
    for s, S in enumerate(SEGS):
        inp(f"dph{s}", [128, S["nq"], 177])
        inp(f"masks{s}", [128, S["nq"] + S["nt2"] + S["nt3"] + S["nn1"]])
        inp(f"xseg{s}", [CIN, S["nt3"], FW])            # x_img slice (zeroed oob)
    # packed f32 constants: [alpha, beta, s_dt2, t_dt2, s_dt3, t_dt3,
    #  s_dn1(2), t_dn1(2), s_dn2(2), t_dn2(2), b_dn3(139)] -> [128, 153]
    inp("consts", [128, 153], dt.float32)
    # conv weights (host-prepped layouts)
    inp("w_dt2", [4, 128, 32])                          # groups (dky,dmx)
    inp("w_dt3", [9, 128, 64])
    inp("w_dn1", [9, 3, 128, 256])                      # tap, icchunk(128,128,64pad) -> 256
    inp("w_dn2", [9, 2, 128, 256])
    inp("w_dn3", [2, 128, 139])

    DBG = {}
    dbg_specs = [] if not debug else [("dbg_t1", [128, SEGS[0]["nq"], 177], dt.bfloat16),
                        ("dbg_dt2o", [32, SEGS[0]["nt2"] + 1, 180], dt.bfloat16),
                        ("dbg_dtc", [64, SEGS[0]["nt3"], 92], dt.bfloat16),
                        ("dbg_n1o", [128, SEGS[0]["nn1"], 92], dt.bfloat16),
                        ("dbg_n2o", [128, SEGS[0]["nout"], 88], dt.bfloat16)]
    for nm, sh, dty in dbg_specs:
        DBG[nm] = nc.dram_tensor(nm, sh, dty, kind="ExternalOutput").ap()
    out_depth = nc.dram_tensor("out_depth", [24 * FW, DD], dt.float32,
                               kind="ExternalOutput").ap()
    out_feat = nc.dram_tensor("out_feat", [24 * FW, CIMG], dt.bfloat16,
                              kind="ExternalOutput").ap()

    # HBM scratch
    scr = {}
    for s, S in enumerate(SEGS):
        scr[f"dt2o{s}"] = nc.dram_tensor(f"dt2o{s}", [32, S["nt2"] + 1, 2, 90], dt.bfloat16).ap()

    RELU = mybir.ActivationFunctionType.Relu
    with tile.TileContext(nc) as tc:
        with tc.tile_pool(name="const", bufs=1) as cpool, \
             tc.tile_pool(name="work", bufs=work_bufs) as wpool, \
             tc.tile_pool(name="big", bufs=1) as bpool, \
             tc.tile_pool(name="psum", bufs=psum_bufs, space="PSUM") as ppool:
            # ---- load packed constants in one DMA ----
            cts = cpool.tile([128, 153], dt.float32, name="cts")
            nc.sync.dma_start(out=cts[:], in_=AP["consts"])
            ct = {"dt1_alpha": cts[:, 0:1], "dt1_beta": cts[:, 1:2],
                  "s_dt2": cts[:, 2:3], "t_dt2": cts[:, 3:4],
                  "s_dt3": cts[:, 4:5], "t_dt3": cts[:, 5:6],
                  "s_dn1": cts[:, 6:8], "t_dn1": cts[:, 8:10],
                  "s_dn2": cts[:, 10:12], "t_dn2": cts[:, 12:14],
                  "b_dn3": cts[:, 14:153]}
            wt = {}
            for nm, pat in [("w_dt2", "g p o -> p g o"),
                            ("w_dt3", "g p o -> p g o"),
                            ("w_dn1", "t i p o -> p (t i) o"),
                            ("w_dn2", "t i p o -> p (t i) o"),
                            ("w_dn3", "g p o -> p g o")]:
                sh = list(AP[nm].shape)
                wt[nm] = cpool.tile([sh[-2], int(np.prod(sh[:-2])), sh[-1]],
                                    dt.bfloat16, tag=nm, name=f'wt_{nm}')
                nc.sync.dma_start(out=wt[nm][:], in_=AP[nm].rearrange(pat))

            feat_sb = {}
            depth_sb = {}
            for s, S in enumerate(SEGS):
                nq, nt2, nt3, nn1, nout = S["nq"], S["nt2"], S["nt3"], S["nn1"], S["nout"]
                # ======== dt1 : affine + relu + row-mask on host-phased d ====
                dph = bpool.tile([128, nq, 177], dt.bfloat16, tag=f"dph{s}")
                for qq in range(0, nq, nq // 4):
                    nqq = min(nq // 4, nq - qq)
                    nc.sync.dma_start(out=dph[:, qq:qq + nqq, :],
                                      in_=AP[f"dph{s}"][:, qq:qq + nqq, :])
                t1 = bpool.tile([128, nq, 177], dt.bfloat16, tag=f"t1{s}")
                mall = wpool.tile([128, nq + nt2 + nt3 + nn1], dt.bfloat16,
                                  tag=f"msk{s}", name="mall")
                nc.sync.dma_start(out=mall[:], in_=AP[f"masks{s}"])
                QCH = nq // 4
                for qq in range(0, nq, QCH):
                    nqq = min(QCH, nq - qq)
                    sl = (slice(None), slice(qq, qq + nqq), slice(None))
                    nc.vector.tensor_scalar(out=t1[sl], in0=dph[sl],
                                            scalar1=ct["dt1_alpha"][:, 0:1],
                                            scalar2=ct["dt1_beta"][:, 0:1],
                                            op0=mybir.AluOpType.mult,
                                            op1=mybir.AluOpType.add)
                    nc.vector.tensor_scalar(out=t1[sl], in0=t1[sl], scalar1=0.0,
                                            scalar2=None, op0=mybir.AluOpType.max)
                    mb = bass.AP(mall.tensor, mall.offset + qq,
                                 [mall.ap[0], [1, nqq], [0, 177]])
                    nc.vector.tensor_tensor(out=t1[sl], in0=t1[sl], in1=mb,
                                            op=mybir.AluOpType.mult)
                    nc.vector.memset(t1[:, qq:qq + nqq, 0:1], 0.0)
                if s == 0 and debug:
                    nc.sync.dma_start(out=DBG["dbg_t1"], in_=t1[:])

                if stages < 2:
                    continue
                # ======== dt2 ========
                o2 = bpool.tile([32, nt2 + 1, 180], dt.bfloat16, tag=f"o2{s}")
                nc.vector.memset(o2[:], 0.0)
                m2 = bass.AP(mall.tensor, mall.offset + nq, [mall.ap[0], [1, nt2]])
                RPP2 = 2
                for q0 in range(0, nt2, RPP2):
                    nr = min(RPP2, nt2 - q0)
                    ps = ppool.tile([32, nr, 176], dt.float32, tag=f"ps{s}", name="ps2")
                    gi = 0
                    for dky in range(2):
                        for dmx in range(2):
                            g = dky * 2 + dmx
                            rhs = bass.AP(
                                t1.tensor, t1.offset + (q0 + dky) * 177 + dmx,
                                [t1.ap[0], [177, nr], [1, 176]])
                            nc.tensor.matmul(ps[:], wt["w_dt2"][:, g, :], rhs,
                                             start=(gi == 0), stop=(gi == 3))
                            gi += 1
                    ev = wpool.tile([32, nr, 176], dt.bfloat16, tag=f"ev2{s}")
                    nc.scalar.activation(ev[:], ps[:], RELU,
                                         bias=ct["t_dt2"][0:32, 0:1],
                                         scale=ct["s_dt2"][0:32, 0:1])
                    mbb = bass.AP(m2.tensor, m2.offset + q0,
                                  [[m2.ap[0][0], 32], [1, nr], [0, 176]])
                    # write col c at (c%2)*90 + c//2 + 1  (phase-split layout)
                    o2dst = bass.AP(o2.tensor, o2.offset + q0 * 180 + 1,
                                    [[o2.ap[0][0], 32], [180, nr],
                                     [1, 88], [90, 2]])
                    nc.vector.tensor_tensor(out=o2dst, in0=ev[:], in1=mbb,
                                            op=mybir.AluOpType.mult)
                nc.sync.dma_start(out=scr[f"dt2o{s}"],
                                  in_=o2.rearrange("p q (b x) -> p q b x", b=2))
                if s == 0 and debug:
                    nc.sync.dma_start(out=DBG["dbg_dt2o"], in_=o2[:])

                if stages < 3:
                    continue
                # ======== dt3 ========
                nry3 = nt3 + 2
                ph3 = bpool.tile([128, nry3, 90], dt.bfloat16, tag=f"ph3{s}")
                sd2 = scr[f"dt2o{s}"]
                for a2 in range(2):
                    for b2 in range(2):
                        pap3 = bass.AP(sd2.tensor,
                                       sd2.offset + a2 * 180 + b2 * 90,
                                       [[(nt2 + 1) * 180, 32],
                                        [2 * 180, nry3], [1, 90]])
                        nc.sync.dma_start(
                            out=ph3[(a2 * 2 + b2) * 32:(a2 * 2 + b2 + 1) * 32],
                            in_=pap3)
                # concat input tile: [64 dt3 | pad] plus x_img tiles
                dtc = bpool.tile([64, nt3, 92], dt.bfloat16, tag=f"dtc{s}")
                nc.vector.memset(dtc[:], 0.0)
                m3 = bass.AP(mall.tensor, mall.offset + nq + nt2,
                             [mall.ap[0], [1, nt3]])
                RPP3 = 4
                for t0 in range(0, nt3, RPP3):
                    nr = min(RPP3, nt3 - t0)
                    ps = ppool.tile([64, nr, 88], dt.float32, tag=f"ps{s}")
                    gi = 0
                    for dky in range(3):
                        for dmx in range(3):
                            g = dky * 3 + dmx
                            rhs = bass.AP(ph3.tensor,
                                          ph3.offset + (t0 + dky) * 90 + dmx,
                                          [ph3.ap[0], [90, nr], [1, 88]])
                            nc.tensor.matmul(ps[:], wt["w_dt3"][:, g, :], rhs,
                                             start=(gi == 0), stop=(gi == 8))
                            gi += 1
                    ev = wpool.tile([64, nr, 88], dt.bfloat16, tag=f"ev3{s}")
                    nc.scalar.activation(ev[:], ps[:], RELU,
                                         bias=ct["t_dt3"][0:64, 0:1],
                                         scale=ct["s_dt3"][0:64, 0:1])
                    mbb = bass.AP(m3.tensor, m3.offset + t0,
                                  [m3.ap[0], [1, nr], [0, 88]])
                    nc.vector.tensor_tensor(out=dtc[:, t0:t0 + nr, 2:90],
                                            in0=ev[:], in1=mbb[0:64],
                                            op=mybir.AluOpType.mult)

                if s == 0 and debug:
                    nc.sync.dma_start(out=DBG["dbg_dtc"], in_=dtc[:])
                if stages < 4:
                    continue
                # ======== dn1 ========
                xs = []
                for g in range(2):
                    xt = bpool.tile([128, nt3, 92], dt.bfloat16, tag=f"x{g}_{s}",
                                     name=f"xseg_t{g}")
                    nc.vector.memset(xt[:], 0.0)
                    nc.sync.dma_start(
                        out=xt[:, :, 2:90],
                        in_=AP[f"xseg{s}"][g * 128:(g + 1) * 128])
                    xs.append(xt)
                mn1 = bass.AP(mall.tensor, mall.offset + nq + nt2 + nt3,
                              [mall.ap[0], [1, nn1]])
                n1o = []
                for g in range(2):
                    t = bpool.tile([128, nn1, 92], dt.bfloat16, tag=f"n1o{g}_{s}")
                    nc.vector.memset(t[:], 0.0)
                    n1o.append(t)
                RPP = 5
                for ocg in range(2):
                    for r0 in range(0, nn1, RPP):
                        nr = min(RPP, nn1 - r0)
                        ps = ppool.tile([128, nr, 88], dt.float32, tag=f"ps{s}")
                        gi = 0
                        for ky in range(3):
                            for kx in range(3):
                                tap = ky * 3 + kx
                                for icc, srcT in enumerate((xs[0], xs[1], dtc)):
                                    kk = 128 if icc < 2 else 64
                                    rhs = bass.AP(
                                        srcT.tensor,
                                        srcT.offset + (r0 + ky + 1) * 92 + kx + 1,
                                        [srcT.ap[0], [92, nr], [1, 88]])
                                    lhs = wt["w_dn1"][0:kk, tap * 3 + icc,
                                                      ocg * 128:(ocg + 1) * 128]
                                    nc.tensor.matmul(ps[:], lhs, rhs,
                                                     start=(gi == 0),
                                                     stop=(gi == 26))
                                    gi += 1
                        ev = wpool.tile([128, nr, 88], dt.bfloat16, tag=f"evn1{s}")
                        nc.scalar.activation(ev[:], ps[:], RELU,
                                             bias=ct["t_dn1"][:, ocg:ocg + 1],
                                             scale=ct["s_dn1"][:, ocg:ocg + 1])
                        mbb = bass.AP(mn1.tensor, mn1.offset + r0,
                                      [mn1.ap[0], [1, nr], [0, 88]])
                        nc.vector.tensor_tensor(
                            out=n1o[ocg][:, r0:r0 + nr, 2:90],
                            in0=ev[:], in1=mbb, op=mybir.AluOpType.mult)

                if s == 0 and debug:
                    nc.sync.dma_start(out=DBG["dbg_n1o"], in_=n1o[0][:])
                if stages < 5:
                    continue
                # ======== dn2 ========
                n2o = []
                for g in range(2):
                    n2o.append(bpool.tile([128, nout, 88], dt.bfloat16,
                                          tag=f"n2o{g}_{s}", name=f"n2o{g}"))
                for ocg in range(2):
                    for r0 in range(0, nout, RPP):
                        nr = min(RPP, nout - r0)
                        ps = ppool.tile([128, nr, 88], dt.float32, tag=f"ps{s}")
                        gi = 0
                        for ky in range(3):
                            for kx in range(3):
                                tap = ky * 3 + kx
                                for icc in range(2):
                                    rhs = bass.AP(
                                        n1o[icc].tensor,
                                        n1o[icc].offset + (r0 + ky) * 92 + kx + 1,
                                        [n1o[icc].ap[0], [92, nr], [1, 88]])
                                    lhs = wt["w_dn2"][:, tap * 2 + icc,
                                                      ocg * 128:(ocg + 1) * 128]
                                    nc.tensor.matmul(ps[:], lhs, rhs,
                                                     start=(gi == 0),
                                                     stop=(gi == 17))
                                    gi += 1
                        ev = wpool.tile([128, nr, 88], dt.bfloat16, tag=f"evn2{s}")
                        nc.scalar.activation(ev[:], ps[:], RELU,
                                             bias=ct["t_dn2"][:, ocg:ocg + 1],
                                             scale=ct["s_dn2"][:, ocg:ocg + 1])
                        nc.vector.tensor_copy(n2o[ocg][:, r0:r0 + nr, :], ev[:])

                if s == 0 and debug:
                    nc.sync.dma_start(out=DBG["dbg_n2o"], in_=n2o[0][:])
                if stages < 6:
                    continue
                # ======== dn3 + softmax + feat ========
                npix = nout * FW
                feat_sb[s] = bpool.tile([128, (npix + 127) // 128, CIMG],
                                        dt.bfloat16, tag=f"feat{s}", name=f"feat_sb{s}")
                depth_sb[s] = bpool.tile([128, (npix + 127) // 128, DD],
                                         dt.float32, tag=f"depth{s}", name=f"depth_sb{s}")
                n2f = [t.rearrange("p a b -> p (a b)") for t in n2o]
                for pc in range((npix + 127) // 128):
                    m = min(128, npix - pc * 128)
                    ps = ppool.tile([m, 139], dt.float32, tag=f"ps{s}")
                    for icc in range(2):
                        nc.tensor.matmul(ps[:], n2f[icc][:, pc * 128:pc * 128 + m],
                                         wt["w_dn3"][:, icc, :],
                                         start=(icc == 0), stop=(icc == 1))
                    # add bias via vector then softmax over first 59
                    lg = wpool.tile([m, 139], dt.float32, tag=f"lg{s}")
                    nc.vector.tensor_tensor(out=lg[:], in0=ps[:],
                                            in1=ct["b_dn3"][0:m],
                                            op=mybir.AluOpType.add)
                    mx = wpool.tile([m, 1], dt.float32, tag=f"mx{s}")
                    nc.vector.reduce_max(mx[:], lg[:, 0:DD],
                                         axis=mybir.AxisListType.X, negate=True)
                    ex = wpool.tile([m, DD], dt.float32, tag=f"ex{s}")
                    nc.scalar.activation(ex[:], lg[:, 0:DD],
                                         mybir.ActivationFunctionType.Exp,
                                         bias=mx[:, 0:1], scale=1.0)
                    sm = wpool.tile([m, 1], dt.float32, tag=f"sm{s}")
                    nc.vector.reduce_sum(sm[:], ex[:], axis=mybir.AxisListType.X)
                    rc = wpool.tile([m, 1], dt.float32, tag=f"rc{s}")
                    nc.vector.reciprocal(rc[:], sm[:])
                    nc.vector.tensor_scalar(out=depth_sb[s][0:m, pc, :], in0=ex[:],
                                            scalar1=rc[:, 0:1], scalar2=None,
                                            op0=mybir.AluOpType.mult)
                    nc.vector.tensor_copy(feat_sb[s][0:m, pc, :],
                                          lg[:, DD:DD + CIMG])

            # DMA outputs: global pix index = seg-A pix then seg-B pix
            for s, S in (enumerate(SEGS) if stages >= 6 else []):
                npix = S["nout"] * FW
                base = 0 if s == 0 else 16 * FW
                nfull = npix // 128
                dsl = out_depth[base:base + nfull * 128].rearrange(
                    "(a p) d -> p a d", p=128)
                nc.sync.dma_start(out=dsl, in_=depth_sb[s][:, 0:nfull, :])
                fsl = out_feat[base:base + nfull * 128].rearrange(
                    "(a p) d -> p a d", p=128)
                nc.sync.dma_start(out=fsl, in_=feat_sb[s][:, 0:nfull, :])
                rem = npix - nfull * 128
                if rem:
                    nc.sync.dma_start(
                        out=out_depth[base + nfull * 128:base + npix],
                        in_=depth_sb[s][0:rem, nfull, :])
                    nc.sync.dma_start(
                        out=out_feat[base + nfull * 128:base + npix],
                        in_=feat_sb[s][0:rem, nfull, :])
    nc.compile()
    return nc


# ------------------------------------------------------------ host helpers
def _host_geometry(rots, trans, intr, post_rots, post_trans):
    import jax
    import jax.numpy as jnp
    with jax.default_device(jax.devices("cpu")[0]):
        f32 = jnp.float32
        ds = jnp.arange(1.0, 60.0, 1.0, dtype=f32)
        xs = jnp.linspace(0.0, IW - 1.0, FW, dtype=f32)
        ys = jnp.linspace(0.0, IH - 1.0, FH, dtype=f32)
        dm = jnp.broadcast_to(ds[:, None, None], (DD, FH, FW))
        xm = jnp.broadcast_to(xs[None, None, :], (DD, FH, FW))
        ym = jnp.broadcast_to(ys[None, :, None], (DD, FH, FW))
        fr = jnp.stack([xm, ym, dm], -1)
        pts = fr[None, None] - jnp.asarray(post_trans)[:, :, None, None, None, :]
        pts = jnp.einsum("bnij,bndhwj->bndhwi",
                         jnp.linalg.inv(jnp.asarray(post_rots)), pts)
        pts = jnp.concatenate([pts[..., :2] * pts[..., 2:3], pts[..., 2:3]], -1)
        comb = jnp.einsum("bnij,bnjk->bnik", jnp.asarray(rots),
                          jnp.linalg.inv(jnp.asarray(intr)))
        pts = jnp.einsum("bnij,bndhwj->bndhwi", comb, pts) \
            + jnp.asarray(trans)[:, :, None, None, None, :]
        lo = jnp.array([XY0, XY0, Z0], dtype=f32)
        dxv = jnp.array([DXY, DXY, DZ], dtype=f32)
        g = ((pts - lo) / dxv).astype(jnp.int32).reshape(-1, 3)
        kept = ((g[:, 0] >= 0) & (g[:, 0] < NX) & (g[:, 1] >= 0) & (g[:, 1] < NX)
                & (g[:, 2] >= 0) & (g[:, 2] < NZ))
        flat = (g[:, 2] * NX + g[:, 0]) * NX + g[:, 1]
        return np.asarray(flat, np.int64), np.asarray(kept)


def _prep_a_inputs(inputs):
    """Build per-core input maps for launch A."""
    d = np.asarray(inputs["d"], np.float32).reshape(N, IH, IW)
    x_img = np.asarray(inputs["x_img"], np.float32)

    # dt1 folded affine: relu(alpha*d + beta), alpha = s*w, beta = s*b + t
    a1 = (inputs["dt1_s"] * inputs["dt1_w"][:, 0, 0, 0]).astype(np.float32)
    b1 = (inputs["dt1_s"] * inputs["dt1_b"] + inputs["dt1_t"]).astype(np.float32)
    cab = np.arange(128)
    dt1_alpha = a1[cab // 16][:, None]
    dt1_beta = b1[cab // 16][:, None]

    def wprep_dt2():
        w = np.asarray(inputs["dt2_w"], np.float32)      # [32,8,5,5]
        out = np.zeros((4, 128, 32), np.float32)
        for ky in range(5):
            for kx in range(5):
                a, dky = ky % 4, ky // 4
                bph, dmx = (kx + 2) % 4, (kx + 2) // 4
                g = dky * 2 + dmx
                rows = (np.arange(8)) * 16 + a * 4 + bph
                out[g, rows, :] = w[:, :, ky, kx].T
        return out.astype(bf16)

    def wprep_dt3():
        w = np.asarray(inputs["dt3_w"], np.float32)      # [64,32,5,5]
        out = np.zeros((9, 128, 64), np.float32)
        for ky in range(5):
            for kx in range(5):
                a, dky = ky % 2, ky // 2
                bph, dmx = kx % 2, (kx + 2) // 2 - 1
                g = dky * 3 + dmx
                rows = (a * 2 + bph) * 32 + np.arange(32)
                out[g, rows, :] = w[:, :, ky, kx].T
        return out.astype(bf16)

    def wprep_3x3(w, icc_sizes):
        O, I = w.shape[0], w.shape[1]
        nic = len(icc_sizes)
        out = np.zeros((9, nic, 128, O), np.float32)
        for ky in range(3):
            for kx in range(3):
                tap = ky * 3 + kx
                ic0 = 0
                for icc, sz in enumerate(icc_sizes):
                    out[tap, icc, 0:sz, :] = w[:, ic0:ic0 + sz, ky, kx].T
                    ic0 += sz
        return out.astype(bf16)

    # NOTE: dn1 input concat order is [dt3(64) | x_img(256)] in the reference;
    # our matmul chunks are (x0:128, x1:128, dt3:64) -> weight cols must match:
    w_dn1_full = np.asarray(inputs["dn1_w"], np.float32)
    w_dn1 = np.zeros((9, 3, 128, 256), np.float32)
    for ky in range(3):
        for kx in range(3):
            tap = ky * 3 + kx
            w_dn1[tap, 0, :, :] = w_dn1_full[:, 64:192, ky, kx].T
            w_dn1[tap, 1, :, :] = w_dn1_full[:, 192:320, ky, kx].T
            w_dn1[tap, 2, 0:64, :] = w_dn1_full[:, 0:64, ky, kx].T
    w_dn1 = w_dn1.astype(bf16)
    w_dn2 = wprep_3x3(np.asarray(inputs["dn2_w"], np.float32), [128, 128])
    w_dn3 = np.asarray(inputs["dn3_w"], np.float32)[:, :, 0, 0]  # [139, 256]
    w_dn3p = np.zeros((2, 128, 139), np.float32)
    w_dn3p[0] = w_dn3[:, 0:128].T
    w_dn3p[1] = w_dn3[:, 128:256].T

    def fold_bias(b, s, t):
        # conv bias b then bn scale/shift: relu(s*(x+b) + t) = relu(s*x + (s*b+t))
        return np.asarray(s, np.float32), np.asarray(s * b + t, np.float32)

    s2, t2 = fold_bias(inputs["dt2_b"], inputs["dt2_s"], inputs["dt2_t"])
    s3, t3 = fold_bias(inputs["dt3_b"], inputs["dt3_s"], inputs["dt3_t"])
    sn1, tn1 = fold_bias(inputs["dn1_b"], inputs["dn1_s"], inputs["dn1_t"])
    sn2, tn2 = fold_bias(inputs["dn2_b"], inputs["dn2_s"], inputs["dn2_t"])
    b_dn3 = np.broadcast_to(np.asarray(inputs["dn3_b"], np.float32)[None, :],
                            (128, 139)).copy()

    consts = np.zeros((128, 153), np.float32)
    consts[:, 0] = dt1_alpha[:, 0]
    consts[:, 1] = dt1_beta[:, 0]
    consts[:, 2] = np.tile(s2, 4)
    consts[:, 3] = np.tile(t2, 4)
    consts[:, 4] = np.tile(s3, 2)
    consts[:, 5] = np.tile(t3, 2)
    consts[:, 6:8] = sn1.reshape(2, 128).T
    consts[:, 8:10] = tn1.reshape(2, 128).T
    consts[:, 10:12] = sn2.reshape(2, 128).T
    consts[:, 12:14] = tn2.reshape(2, 128).T
    consts[:, 14:153] = b_dn3
    shared = dict(
        consts=consts,
        w_dt2=wprep_dt2(), w_dt3=wprep_dt3(), w_dn1=w_dn1, w_dn2=w_dn2,
        w_dn3=w_dn3p.astype(bf16),
    )

    maps = []
    for c in range(NCORES):
        m = dict(shared)
        for s, (cam, h0) in enumerate([SEG_A[c], SEG_B[c]]):
            S = SEGS[s]
            d0 = 8 * h0 - 34
            dseg = np.zeros((S["nd"], 712), np.float32)
            lo, hi = max(0, d0), min(IH, d0 + S["nd"])
            if hi > lo:
                dseg[lo - d0:hi - d0, 4:708] = d[cam, lo:hi]
            nq = S["nq"]
            ph = dseg.reshape(nq, 4, 178, 4)[:, :, :177, :]     # ry a rx b
            ph = ph.transpose(1, 3, 0, 2)                        # a b ry rx
            m[f"dph{s}"] = np.broadcast_to(
                ph[None], (8, 4, 4, nq, 177)).reshape(128, nq, 177).astype(bf16)
            # dmask: partition (c8,a,b) x ry -> valid(4ry+a)
            ry = np.arange(S["nd"] // 4)
            rows = 4 * ry[None, :] + (cab[:, None] // 4) % 4
            dmask = (((rows + d0) >= 0) & ((rows + d0) < IH))
            q0, t0, r0 = 2 * h0 - 8, h0 - 3, h0 - 1
            qr = np.arange(S["nt2"]) + q0
            m2m = np.broadcast_to(((qr >= 0) & (qr < 64))[None, :],
                                  (128, S["nt2"]))
            tr = np.arange(S["nt3"]) + t0
            m3m = np.broadcast_to(((tr >= 0) & (tr < FH))[None, :],
                                  (128, S["nt3"]))
            rr = np.arange(S["nn1"]) + r0
            mn1m = np.broadcast_to(((rr >= 0) & (rr < FH))[None, :],
                                   (128, S["nn1"]))
            m[f"masks{s}"] = np.concatenate(
                [dmask, m2m, m3m, mn1m], axis=1).astype(bf16)
            xseg = np.zeros((CIN, S["nt3"], FW), np.float32)
            lo2, hi2 = max(0, t0), min(FH, t0 + S["nt3"])
            if hi2 > lo2:
                xseg[:, lo2 - t0:hi2 - t0, :] = x_img[cam, :, lo2:hi2, :]
            m[f"xseg{s}"] = xseg.astype(bf16)
        maps.append(m)
    return maps


# ---------------------------------------------------------------- launch B
def build_launch_b(W):
    """W windows x QV chunks of 128 points; scatter-sum into virtual windows."""
    nc = bacc.Bacc("TRN2", target_bir_lowering=False, debug=False,
                   num_devices=NCORES)
    NCH = W * QV
    pb = nc.dram_tensor("pb", [128, NCH, CIMG], dt.bfloat16,
                        kind="ExternalInput").ap()
    offv = nc.dram_tensor("offv", [128, NCH], dt.float32,
                          kind="ExternalInput").ap()
    depv = nc.dram_tensor("depv", [128, NCH], dt.float32,
                          kind="ExternalInput").ap()
    povirt = nc.dram_tensor("povirt", [128, W, CIMG], dt.float32,
                            kind="ExternalOutput").ap()
    BQ = 32                     # chunks per input batch
    BW = 8                      # windows per output batch
    with tile.TileContext(nc) as tc:
        with tc.tile_pool(name="const", bufs=1) as cpool,              tc.tile_pool(name="io", bufs=4) as iop,              tc.tile_pool(name="g", bufs=8) as gp,              tc.tile_pool(name="ps", bufs=2, space="PSUM") as pp:
            iota4 = cpool.tile([128, 1, 128], dt.bfloat16, name="iota4")
            nc.gpsimd.iota(iota4[:], pattern=[[0, 1], [1, 128]], base=0,
                           channel_multiplier=0,
                           allow_small_or_imprecise_dtypes=True)
            offt = cpool.tile([128, NCH], dt.float32, name="offt")
            nc.sync.dma_start(out=offt[:], in_=offv)
            dept = cpool.tile([128, NCH], dt.float32, name="dept")
            nc.sync.dma_start(out=dept[:], in_=depv)
            for w0 in range(0, W, BW):
                nw = min(BW, W - w0)
                ot = iop.tile([128, BW, CIMG], dt.float32, tag="ot", name="ot")
                for wi in range(nw):
                    w = w0 + wi
                    k0 = w * QV
                    if k0 % BQ == 0:
                        pbt = iop.tile([128, BQ, CIMG], dt.bfloat16,
                                       tag="pbt", name="pbt")
                        nb = min(BQ, NCH - k0)
                        nc.sync.dma_start(out=pbt[:, 0:nb, :],
                                          in_=pb[:, k0:k0 + nb, :])
                    ps = pp.tile([128, CIMG], dt.float32, tag="ps", name="ps")
                    for q in range(QV):
                        k = k0 + q
                        g = gp.tile([128, 128], dt.bfloat16, tag="g", name="g")
                        nc.vector.tensor_scalar(
                            out=g[:], in0=iota4[:, 0, :], scalar1=offt[:, k:k + 1],
                            scalar2=dept[:, k:k + 1],
                            op0=mybir.AluOpType.is_equal,
                            op1=mybir.AluOpType.mult)
                        nc.tensor.matmul(ps[:], g[:], pbt[:, k % BQ, :],
                                         start=(q == 0), stop=(q == QV - 1))
                    nc.scalar.activation(ot[:, wi, :], ps[:],
                                         mybir.ActivationFunctionType.Copy)
                nc.sync.dma_start(out=povirt[:, w0:w0 + nw, :],
                                  in_=ot[:, 0:nw, :])
    nc.compile()
    return nc


# ---------------------------------------------------------------- launch C
C_OUT_ROWS = 23              # ds2-out rows per core (8*23 = 184 >= 180)


def build_launch_c():
    nc = bacc.Bacc("TRN2", target_bir_lowering=False, debug=False,
                   num_devices=NCORES)
    NR1 = C_OUT_ROWS + 2                         # ds1-out rows incl halo (25)
    NRP = 2 * NR1 + 1                            # pooled rows needed (51)
    slab = nc.dram_tensor("slab", [CIMG, NRP, 362], dt.bfloat16,
                          kind="ExternalInput").ap()
    m1 = nc.dram_tensor("m1", [128, NR1], dt.bfloat16, kind="ExternalInput").ap()
    wd1 = nc.dram_tensor("wd1", [9, CIMG, CIMG], dt.bfloat16,
                         kind="ExternalInput").ap()
    wd2 = nc.dram_tensor("wd2", [9, CIMG, CIMG], dt.bfloat16,
                         kind="ExternalInput").ap()
    sb1 = nc.dram_tensor("sb1", [CIMG, 2], dt.float32, kind="ExternalInput").ap()
    sb2 = nc.dram_tensor("sb2", [CIMG, 2], dt.float32, kind="ExternalInput").ap()
    yout = nc.dram_tensor("yout", [CIMG, C_OUT_ROWS, 180], dt.float32,
                          kind="ExternalOutput").ap()
    RELU = mybir.ActivationFunctionType.Relu
    with tile.TileContext(nc) as tc:
        with tc.tile_pool(name="const", bufs=1) as cpool,              tc.tile_pool(name="work", bufs=2) as wp,              tc.tile_pool(name="big", bufs=1) as bp,              tc.tile_pool(name="ps", bufs=3, space="PSUM") as pp:
            slabt = bp.tile([CIMG, NRP, 362], dt.bfloat16, name="slabt")
            for rr in range(0, NRP, 13):
                nrr = min(13, NRP - rr)
                nc.sync.dma_start(out=slabt[:, rr:rr + nrr, :],
                                  in_=slab[:, rr:rr + nrr, :])
            w1 = cpool.tile([CIMG, 9, CIMG], dt.bfloat16, name="w1")
            nc.sync.dma_start(out=w1[:], in_=wd1.rearrange("t p o -> p t o"))
            w2 = cpool.tile([CIMG, 9, CIMG], dt.bfloat16, name="w2")
            nc.sync.dma_start(out=w2[:], in_=wd2.rearrange("t p o -> p t o"))
            sb1t = cpool.tile([CIMG, 2], dt.float32, name="sb1t")
            nc.sync.dma_start(out=sb1t[:], in_=sb1)
            sb2t = cpool.tile([CIMG, 2], dt.float32, name="sb2t")
            nc.sync.dma_start(out=sb2t[:], in_=sb2)
            m1t = wp.tile([128, NR1], dt.bfloat16, name="m1t")
            nc.sync.dma_start(out=m1t[:], in_=m1)
            h1 = bp.tile([CIMG, NR1, 182], dt.bfloat16, name="h1")
            nc.vector.memset(h1[:, :, 0:1], 0.0)
            nc.vector.memset(h1[:, :, 181:182], 0.0)
            # ds1: stride-2 3x3; out row t reads slab rows 2t..2t+2 (slab row 0
            # = pooled row 2o0-3, so out row t (global o0-1+t) reads
            # 2(o0-1+t)-1..+1 - (2o0-3) = 2t..2t+2); col c reads 2c..2c+2
            RP = 2
            for t0 in range(0, NR1, RP):
                nr = min(RP, NR1 - t0)
                ps = pp.tile([CIMG, nr, 180], dt.float32, tag="ps1", name="ps")
                gi = 0
                for ky in range(3):
                    for kx in range(3):
                        rhs = bass.AP(slabt.tensor,
                                      slabt.offset + (2 * t0 + ky) * 362 + kx,
                                      [slabt.ap[0], [2 * 362, nr], [2, 180]])
                        nc.tensor.matmul(ps[:], w1[:, ky * 3 + kx, :], rhs,
                                         start=(gi == 0), stop=(gi == 8))
                        gi += 1
                ev = wp.tile([CIMG, nr, 180], dt.bfloat16, tag="ev", name="ev")
                nc.scalar.activation(ev[:], ps[:], RELU, bias=sb1t[:, 1:2],
                                     scale=sb1t[:, 0:1])
                mbb = bass.AP(m1t.tensor, m1t.offset + t0,
                              [[m1t.ap[0][0], CIMG], [1, nr], [0, 180]])
                nc.vector.tensor_tensor(out=h1[:, t0:t0 + nr, 1:181],
                                        in0=ev[:], in1=mbb,
                                        op=mybir.AluOpType.mult)
            # ds2: 3x3 pad 1: out row o reads h1 rows o..o+2, col c: c..c+2
            yo = bp.tile([CIMG, C_OUT_ROWS, 180], dt.float32, name="yo")
            for o0 in range(0, C_OUT_ROWS, RP):
                nr = min(RP, C_OUT_ROWS - o0)
                ps = pp.tile([CIMG, nr, 180], dt.float32, tag="ps2", name="ps")
                gi = 0
                for ky in range(3):
                    for kx in range(3):
                        rhs = bass.AP(h1.tensor,
                                      h1.offset + (o0 + ky) * 182 + kx,
                                      [h1.ap[0], [182, nr], [1, 180]])
                        nc.tensor.matmul(ps[:], w2[:, ky * 3 + kx, :], rhs,
                                         start=(gi == 0), stop=(gi == 8))
                        gi += 1
                nc.scalar.activation(yo[:, o0:o0 + nr, :], ps[:], RELU,
                                     bias=sb2t[:, 1:2], scale=sb2t[:, 0:1])
                nc.sync.dma_start(out=yout[:, o0:o0 + nr, :],
                                  in_=yo[:, o0:o0 + nr, :])
    nc.compile()
    return nc


_CACHE = {}


def run_launch_a(inputs):
    if "A" not in _CACHE:
        _CACHE["A"] = build_launch_a()
    nc = _CACHE["A"]
    maps = _prep_a_inputs(inputs)
    res = run_bass_kernel_spmd(nc, maps, list(range(NCORES)))
    depth = np.zeros((NPIX, DD), np.float32)
    feat = np.zeros((NPIX, CIMG), np.float32)
    for c in range(NCORES):
        r = res.results[c]
        for s, (cam, h0) in enumerate([SEG_A[c], SEG_B[c]]):
            S = SEGS[s]
            npix = S["nout"] * FW
            base = (cam * FH + h0) * FW
            off = 0 if s == 0 else 16 * FW
            depth[base:base + npix] = r["out_depth"][off:off + npix]
            feat[base:base + npix] = r["out_feat"][off:off + npix].astype(np.float32)
    return depth, feat


def _build_schedule(flat, kept):
    """Sort kept points by (core, local voxel); emit fixed-quota virtual
    windows of QV*128 points with vox-span < 128. Returns per-core schedule
    dicts + W (max window count, rounded to 8)."""
    pts = np.arange(NPTS)
    rem = pts % (DD * FH * FW)
    d_i = rem // (FH * FW)
    col = (pts // (DD * FH * FW)) * (FH * FW) + rem % (FH * FW)
    vox = flat
    vx = (vox // NX).astype(np.int32)

    keep_idx = np.where(kept)[0]
    cnt = np.bincount(vx[keep_idx], minlength=NX)
    order = np.argsort(-cnt, kind="stable")
    core_of_row = np.zeros(NX, np.int32)
    load = np.zeros(NCORES, np.int64)
    for r in order:
        c = int(np.argmin(load))
        core_of_row[r] = c
        load[c] += cnt[r]

    row_rank = np.zeros(NX, np.int64)
    rows_of = []
    for c in range(NCORES):
        rows = np.where(core_of_row == c)[0]
        rows_of.append(rows)
        row_rank[rows] = np.arange(len(rows))

    schedules = []
    for c in range(NCORES):
        sel = keep_idx[core_of_row[vx[keep_idx]] == c]
        vloc = row_rank[vx[sel]] * NX + (vox[sel] % NX)
        o = np.argsort(vloc, kind="stable")
        sel, vloc = sel[o], vloc[o]
        win = []                      # (start, end, base)
        i, n = 0, len(sel)
        while i < n:
            base = vloc[i]
            j = min(i + QV * 128, n)
            hi = np.searchsorted(vloc, base + 128, "left")
            j = min(j, hi)
            win.append((i, j, base))
            i = j
        schedules.append(dict(sel=sel, vloc=vloc, win=win, col=col[sel],
                              d_i=d_i[sel], rows=rows_of[c]))
    W = max(len(s["win"]) for s in schedules)
    W = (W + 7) // 8 * 8
    return schedules, W


def _prep_b_inputs(schedules, W, depth_rows, featflat_bf):
    maps = []
    NCH = W * QV
    for sch in schedules:
        pb = np.zeros((128, NCH, CIMG), bf16)
        offv = np.zeros((128, NCH), np.float32)
        depv = np.zeros((128, NCH), np.float32)  # cast to bf16 at the end
        col, d_i, vloc = sch["col"], sch["d_i"], sch["vloc"]
        dvals = depth_rows[col, d_i]
        for w, (i, j, base) in enumerate(sch["win"]):
            L = j - i
            nch = (L + 127) // 128
            gath = featflat_bf[col[i:j]]
            for q in range(nch):
                lo, hi = q * 128, min((q + 1) * 128, L)
                k = w * QV + q
                pb[0:hi - lo, k] = gath[lo:hi]
                offv[0:hi - lo, k] = vloc[i + lo:i + hi] - base
                depv[0:hi - lo, k] = dvals[i + lo:i + hi]
        maps.append(dict(pb=pb, offv=offv, depv=depv))
    return maps


def _prep_c_inputs(inputs, pooled_t):
    """pooled_t: [CIMG, 360, 360] f32 -> per-core slabs + masks + weights."""
    NR1 = C_OUT_ROWS + 2
    NRP = 2 * NR1 + 1
    w1 = np.asarray(inputs["ds1_w"], np.float32)
    w2 = np.asarray(inputs["ds2_w"], np.float32)
    wd1 = np.stack([w1[:, :, ky, kx].T for ky in range(3) for kx in range(3)])
    wd2 = np.stack([w2[:, :, ky, kx].T for ky in range(3) for kx in range(3)])
    sb1 = np.stack([np.asarray(inputs["ds1_s"], np.float32),
                    np.asarray(inputs["ds1_t"], np.float32)], 1)
    sb2 = np.stack([np.asarray(inputs["ds2_s"], np.float32),
                    np.asarray(inputs["ds2_t"], np.float32)], 1)
    shared = dict(wd1=wd1.astype(bf16), wd2=wd2.astype(bf16), sb1=sb1, sb2=sb2)
    maps = []
    pt_bf = pooled_t.astype(bf16)
    for c in range(NCORES):
        o0g = C_OUT_ROWS * c
        p0 = 2 * o0g - 3
        slab = np.zeros((CIMG, NRP, 362), bf16)
        lo, hi = max(0, p0), min(NX, p0 + NRP)
        if hi > lo:
            slab[:, lo - p0:hi - p0, 1:361] = pt_bf[:, lo:hi, :]
        t1g = np.arange(NR1) + (o0g - 1)
        m1 = np.broadcast_to(((t1g >= 0) & (t1g < 180))[None, :],
                             (128, NR1)).astype(bf16)
        maps.append(dict(shared, slab=slab, m1=np.ascontiguousarray(m1)))
    return maps


def kernel(**inputs):
    inputs = {k: np.asarray(v) for k, v in inputs.items()}
    flat, kept = _host_geometry(inputs["cam2lidar_rots"],
                                inputs["cam2lidar_trans"], inputs["intrins"],
                                inputs["post_rots"], inputs["post_trans"])
    depth_rows, feat_rows = run_launch_a(inputs)
    featflat_bf = feat_rows.astype(bf16)

    schedules, W = _build_schedule(flat, kept)
    key = ("B", W)
    if key not in _CACHE:
        _CACHE[key] = build_launch_b(W)
    bmaps = _prep_b_inputs(schedules, W, depth_rows, featflat_bf)
    res_b = run_bass_kernel_spmd(_CACHE[key], bmaps, list(range(NCORES)))

    pooled = np.zeros((NX * NX, CIMG), np.float32)
    for c, sch in enumerate(schedules):
        virt = res_b.results[c]["povirt"].transpose(1, 0, 2)  # -> [W, 128, C]
        rows_arr = sch["rows"]
        nloc = len(rows_arr) * NX
        for w, (i, j, base) in enumerate(sch["win"]):
            span = min(128, nloc - base)
            lidx = base + np.arange(span)
            ridx = rows_arr[lidx // NX] * NX + (lidx % NX)
            pooled[ridx] += virt[w][:span]
    pooled_t = np.ascontiguousarray(
        pooled.reshape(NX, NX, CIMG).transpose(2, 0, 1))

    if "C" not in _CACHE:
        _CACHE["C"] = build_launch_c()
    cmaps = _prep_c_inputs(inputs, pooled_t)
    res_c = run_bass_kernel_spmd(_CACHE["C"], cmaps, list(range(NCORES)))
    out = np.zeros((1, CIMG, 180, 180), np.float32)
    for c in range(NCORES):
        o0g = C_OUT_ROWS * c
        nr = min(C_OUT_ROWS, 180 - o0g)
        if nr > 0:
            out[0, :, o0g:o0g + nr, :] = res_c.results[c]["yout"][:, 0:nr, :]
    return out



# revision 4
# speedup vs baseline: 1.2403x; 1.2403x over previous
"""DepthLSSTransform Trainium kernel: 3 SPMD launches over 8 NeuronCores.

Launch A: per-camera conv pipeline (dtransform + depthnet + softmax) on
          24-row bands (one 16-row + one 8-row segment per core).
Launch B: bev_pool segment-sum via one-hot matmuls over a host-built
          virtual-window schedule (sorted-by-voxel points).
Launch C: BEV downsample convs, spatially sharded.
Host: geometry/voxel indices, scheduling, gathers, folds (orchestration).
"""
import numpy as np
import ml_dtypes

import concourse.bass as bass
import concourse.tile as tile
from concourse import bacc, mybir
from concourse.bass_utils import run_bass_kernel_spmd

dt = mybir.dt
bf16 = ml_dtypes.bfloat16

# ---- problem constants (hardcoded per contract) ----
B, N = 1, 6
CIN, CIMG, DD = 256, 80, 59
FH, FW, IH, IW = 32, 88, 256, 704
XY0, DXY, NX = -54.0, 0.3, 360
Z0, DZ, NZ = -10.0, 20.0, 1
NPTS = N * DD * FH * FW
NPIX = N * FH * FW
NCORES = 8
QV = 4                      # chunks of 128 points per virtual window

# per-core segments: (camera, h0) for seg A (16 rows) and seg B (8 rows)
SEG_A = [(0, 0), (1, 0), (1, 16), (2, 16), (3, 0), (4, 0), (4, 16), (5, 16)]
SEG_B = [(0, 16), (0, 24), (2, 0), (2, 8), (3, 16), (3, 24), (5, 0), (5, 8)]
# band pixel ranges in global row order (row = n*32 + h)
ROWS_OF_CORE = [[(SEG_A[c][0] * FH + SEG_A[c][1] + r) for r in range(16)] +
                [(SEG_B[c][0] * FH + SEG_B[c][1] + r) for r in range(8)]
                for c in range(NCORES)]

# segment geometry: rows16 segment: d rows [8h0-34, 8h0+158) (192), dt2 out
# rows [2h0-8, 2h0+39) (47), dt3 [h0-3, h0+19) (22), dn1 [h0-1, h0+17) (18)
SEGS = [dict(nout=16, nd=192, nq=48, nt2=47, nt3=22, nn1=18),
        dict(nout=8, nd=128, nq=32, nt2=31, nt3=14, nn1=10)]


def _seg_ranges(h0, S):
    return dict(d0=8 * h0 - 34, q0=2 * h0 - 8, t0=h0 - 3, r0=h0 - 1, o0=h0)


# ---------------------------------------------------------------- launch A
def build_launch_a(debug=False, psum_bufs=3, work_bufs=3, stages=9):
    nc = bacc.Bacc("TRN2", target_bir_lowering=False, debug=False,
                   num_devices=NCORES)
    AP = {}

    def inp(name, shape, dtype=dt.bfloat16):
        AP[name] = nc.dram_tensor(name, shape, dtype, kind="ExternalInput").ap()
        return AP[name]

    # per segment inputs (s = 0: 16-row, 1: 8-row)
    for s, S in enumerate(SEGS):
        inp(f"dph{s}", [128, S["nq"], 177])
        inp(f"masks{s}", [128, S["nq"] + S["nt2"] + S["nt3"] + S["nn1"]])
        inp(f"xseg{s}", [CIN, S["nt3"], FW])            # x_img slice (zeroed oob)
    # packed f32 constants: [alpha, beta, s_dt2, t_dt2, s_dt3, t_dt3,
    #  s_dn1(2), t_dn1(2), s_dn2(2), t_dn2(2), b_dn3(139)] -> [128, 153]
    inp("consts", [128, 153], dt.float32)
    # conv weights (host-prepped layouts)
    inp("w_dt2", [4, 128, 32])                          # groups (dky,dmx)
    inp("w_dt3", [9, 128, 64])
    inp("w_dn1", [9, 3, 128, 256])                      # tap, icchunk(128,128,64pad) -> 256
    inp("w_dn2", [9, 2, 128, 256])
    inp("w_dn3", [2, 128, 139])

    DBG = {}
    dbg_specs = [] if not debug else [("dbg_t1", [128, SEGS[0]["nq"], 177], dt.bfloat16),
                        ("dbg_dt2o", [32, SEGS[0]["nt2"] + 1, 180], dt.bfloat16),
                        ("dbg_dtc", [64, SEGS[0]["nt3"], 92], dt.bfloat16),
                        ("dbg_n1o", [128, SEGS[0]["nn1"], 92], dt.bfloat16),
                        ("dbg_n2o", [128, SEGS[0]["nout"], 88], dt.bfloat16)]
    for nm, sh, dty in dbg_specs:
        DBG[nm] = nc.dram_tensor(nm, sh, dty, kind="ExternalOutput").ap()
    out_depth = nc.dram_tensor("out_depth", [24 * FW, DD], dt.float32,
                               kind="ExternalOutput").ap()
    out_feat = nc.dram_tensor("out_feat", [24 * FW, CIMG], dt.bfloat16,
                              kind="ExternalOutput").ap()

    # HBM scratch
    scr = {}
    for s, S in enumerate(SEGS):
        scr[f"dt2o{s}"] = nc.dram_tensor(f"dt2o{s}", [32, S["nt2"] + 1, 2, 90], dt.bfloat16).ap()

    RELU = mybir.ActivationFunctionType.Relu
    with tile.TileContext(nc) as tc:
        with tc.tile_pool(name="const", bufs=1) as cpool, \
             tc.tile_pool(name="work", bufs=work_bufs) as wpool, \
             tc.tile_pool(name="big", bufs=1) as bpool, \
             tc.tile_pool(name="psum", bufs=psum_bufs, space="PSUM") as ppool:
            # ---- load packed constants in one DMA ----
            cts = cpool.tile([128, 153], dt.float32, name="cts")
            nc.sync.dma_start(out=cts[:], in_=AP["consts"])
            ct = {"dt1_alpha": cts[:, 0:1], "dt1_beta": cts[:, 1:2],
                  "s_dt2": cts[:, 2:3], "t_dt2": cts[:, 3:4],
                  "s_dt3": cts[:, 4:5], "t_dt3": cts[:, 5:6],
                  "s_dn1": cts[:, 6:8], "t_dn1": cts[:, 8:10],
                  "s_dn2": cts[:, 10:12], "t_dn2": cts[:, 12:14],
                  "b_dn3": cts[:, 14:153]}
            wt = {}
            for nm, pat in [("w_dt2", "g p o -> p g o"),
                            ("w_dt3", "g p o -> p g o"),
                            ("w_dn1", "t i p o -> p (t i) o"),
                            ("w_dn2", "t i p o -> p (t i) o"),
                            ("w_dn3", "g p o -> p g o")]:
                sh = list(AP[nm].shape)
                wt[nm] = cpool.tile([sh[-2], int(np.prod(sh[:-2])), sh[-1]],
                                    dt.bfloat16, tag=nm, name=f'wt_{nm}')
                nc.sync.dma_start(out=wt[nm][:], in_=AP[nm].rearrange(pat))

            feat_sb = {}
            depth_sb = {}
            for s, S in enumerate(SEGS):
                nq, nt2, nt3, nn1, nout = S["nq"], S["nt2"], S["nt3"], S["nn1"], S["nout"]
                # ======== dt1 : affine + relu + row-mask on host-phased d ====
                dph = bpool.tile([128, nq, 177], dt.bfloat16, tag=f"dph{s}")
                for qq in range(0, nq, nq // 4):
                    nqq = min(nq // 4, nq - qq)
                    nc.sync.dma_start(out=dph[:, qq:qq + nqq, :],
                                      in_=AP[f"dph{s}"][:, qq:qq + nqq, :])
                t1 = bpool.tile([128, nq, 177], dt.bfloat16, tag=f"t1{s}")
                mall = wpool.tile([128, nq + nt2 + nt3 + nn1], dt.bfloat16,
                                  tag=f"msk{s}", name="mall")
                nc.sync.dma_start(out=mall[:], in_=AP[f"masks{s}"])
                QCH = nq // 4
                for qq in range(0, nq, QCH):
                    nqq = min(QCH, nq - qq)
                    sl = (slice(None), slice(qq, qq + nqq), slice(None))
                    nc.vector.tensor_scalar(out=t1[sl], in0=dph[sl],
                                            scalar1=ct["dt1_alpha"][:, 0:1],
                                            scalar2=ct["dt1_beta"][:, 0:1],
                                            op0=mybir.AluOpType.mult,
                                            op1=mybir.AluOpType.add)
                    nc.vector.tensor_scalar(out=t1[sl], in0=t1[sl], scalar1=0.0,
                                            scalar2=None, op0=mybir.AluOpType.max)
                    mb = bass.AP(mall.tensor, mall.offset + qq,
                                 [mall.ap[0], [1, nqq], [0, 177]])
                    nc.vector.tensor_tensor(out=t1[sl], in0=t1[sl], in1=mb,
                                            op=mybir.AluOpType.mult)
                    nc.vector.memset(t1[:, qq:qq + nqq, 0:1], 0.0)
                if s == 0 and debug:
                    nc.sync.dma_start(out=DBG["dbg_t1"], in_=t1[:])

                if stages < 2:
                    continue
                # ======== dt2 ========
                o2 = bpool.tile([32, nt2 + 1, 180], dt.bfloat16, tag=f"o2{s}")
                nc.vector.memset(o2[:], 0.0)
                m2 = bass.AP(mall.tensor, mall.offset + nq, [mall.ap[0], [1, nt2]])
                RPP2 = 2
                for q0 in range(0, nt2, RPP2):
                    nr = min(RPP2, nt2 - q0)
                    ps = ppool.tile([32, nr, 176], dt.float32, tag=f"ps{s}", name="ps2")
                    gi = 0
                    for dky in range(2):
                        for dmx in range(2):
                            g = dky * 2 + dmx
                            rhs = bass.AP(
                                t1.tensor, t1.offset + (q0 + dky) * 177 + dmx,
                                [t1.ap[0], [177, nr], [1, 176]])
                            nc.tensor.matmul(ps[:], wt["w_dt2"][:, g, :], rhs,
                                             start=(gi == 0), stop=(gi == 3))
                            gi += 1
                    ev = wpool.tile([32, nr, 176], dt.bfloat16, tag=f"ev2{s}")
                    nc.scalar.activation(ev[:], ps[:], RELU,
                                         bias=ct["t_dt2"][0:32, 0:1],
                                         scale=ct["s_dt2"][0:32, 0:1])
                    mbb = bass.AP(m2.tensor, m2.offset + q0,
                                  [[m2.ap[0][0], 32], [1, nr], [0, 176]])
                    # write col c at (c%2)*90 + c//2 + 1  (phase-split layout)
                    o2dst = bass.AP(o2.tensor, o2.offset + q0 * 180 + 1,
                                    [[o2.ap[0][0], 32], [180, nr],
                                     [1, 88], [90, 2]])
                    nc.vector.tensor_tensor(out=o2dst, in0=ev[:], in1=mbb,
                                            op=mybir.AluOpType.mult)
                nc.sync.dma_start(out=scr[f"dt2o{s}"],
                                  in_=o2.rearrange("p q (b x) -> p q b x", b=2))
                if s == 0 and debug:
                    nc.sync.dma_start(out=DBG["dbg_dt2o"], in_=o2[:])

                if stages < 3:
                    continue
                # ======== dt3 ========
                nry3 = nt3 + 2
                ph3 = bpool.tile([128, nry3, 90], dt.bfloat16, tag=f"ph3{s}")
                sd2 = scr[f"dt2o{s}"]
                for a2 in range(2):
                    for b2 in range(2):
                        pap3 = bass.AP(sd2.tensor,
                                       sd2.offset + a2 * 180 + b2 * 90,
                                       [[(nt2 + 1) * 180, 32],
                                        [2 * 180, nry3], [1, 90]])
                        nc.sync.dma_start(
                            out=ph3[(a2 * 2 + b2) * 32:(a2 * 2 + b2 + 1) * 32],
                            in_=pap3)
                # concat input tile: [64 dt3 | pad] plus x_img tiles
                dtc = bpool.tile([64, nt3, 92], dt.bfloat16, tag=f"dtc{s}")
                nc.vector.memset(dtc[:], 0.0)
                m3 = bass.AP(mall.tensor, mall.offset + nq + nt2,
                             [mall.ap[0], [1, nt3]])
                RPP3 = 4
                for t0 in range(0, nt3, RPP3):
                    nr = min(RPP3, nt3 - t0)
                    ps = ppool.tile([64, nr, 88], dt.float32, tag=f"ps{s}")
                    gi = 0
                    for dky in range(3):
                        for dmx in range(3):
                            g = dky * 3 + dmx
                            rhs = bass.AP(ph3.tensor,
                                          ph3.offset + (t0 + dky) * 90 + dmx,
                                          [ph3.ap[0], [90, nr], [1, 88]])
                            nc.tensor.matmul(ps[:], wt["w_dt3"][:, g, :], rhs,
                                             start=(gi == 0), stop=(gi == 8))
                            gi += 1
                    ev = wpool.tile([64, nr, 88], dt.bfloat16, tag=f"ev3{s}")
                    nc.scalar.activation(ev[:], ps[:], RELU,
                                         bias=ct["t_dt3"][0:64, 0:1],
                                         scale=ct["s_dt3"][0:64, 0:1])
                    mbb = bass.AP(m3.tensor, m3.offset + t0,
                                  [m3.ap[0], [1, nr], [0, 88]])
                    nc.vector.tensor_tensor(out=dtc[:, t0:t0 + nr, 2:90],
                                            in0=ev[:], in1=mbb[0:64],
                                            op=mybir.AluOpType.mult)

                if s == 0 and debug:
                    nc.sync.dma_start(out=DBG["dbg_dtc"], in_=dtc[:])
                if stages < 4:
                    continue
                # ======== dn1 ========
                xs = []
                for g in range(2):
                    xt = bpool.tile([128, nt3, 92], dt.bfloat16, tag=f"x{g}_{s}",
                                     name=f"xseg_t{g}")
                    nc.vector.memset(xt[:], 0.0)
                    nc.sync.dma_start(
                        out=xt[:, :, 2:90],
                        in_=AP[f"xseg{s}"][g * 128:(g + 1) * 128])
                    xs.append(xt)
                mn1 = bass.AP(mall.tensor, mall.offset + nq + nt2 + nt3,
                              [mall.ap[0], [1, nn1]])
                n1o = []
                for g in range(2):
                    t = bpool.tile([128, nn1, 92], dt.bfloat16, tag=f"n1o{g}_{s}")
                    nc.vector.memset(t[:], 0.0)
                    n1o.append(t)
                RPP = 5
                for ocg in range(2):
                    for r0 in range(0, nn1, RPP):
                        nr = min(RPP, nn1 - r0)
                        ps = ppool.tile([128, nr, 88], dt.float32, tag=f"ps{s}")
                        gi = 0
                        for ky in range(3):
                            for kx in range(3):
                                tap = ky * 3 + kx
                                for icc, srcT in enumerate((xs[0], xs[1], dtc)):
                                    kk = 128 if icc < 2 else 64
                                    rhs = bass.AP(
                                        srcT.tensor,
                                        srcT.offset + (r0 + ky + 1) * 92 + kx + 1,
                                        [srcT.ap[0], [92, nr], [1, 88]])
                                    lhs = wt["w_dn1"][0:kk, tap * 3 + icc,
                                                      ocg * 128:(ocg + 1) * 128]
                                    nc.tensor.matmul(ps[:], lhs, rhs,
                                                     start=(gi == 0),
                                                     stop=(gi == 26))
                                    gi += 1
                        ev = wpool.tile([128, nr, 88], dt.bfloat16, tag=f"evn1{s}")
                        nc.scalar.activation(ev[:], ps[:], RELU,
                                             bias=ct["t_dn1"][:, ocg:ocg + 1],
                                             scale=ct["s_dn1"][:, ocg:ocg + 1])
                        mbb = bass.AP(mn1.tensor, mn1.offset + r0,
                                      [mn1.ap[0], [1, nr], [0, 88]])
                        nc.vector.tensor_tensor(
                            out=n1o[ocg][:, r0:r0 + nr, 2:90],
                            in0=ev[:], in1=mbb, op=mybir.AluOpType.mult)

                if s == 0 and debug:
                    nc.sync.dma_start(out=DBG["dbg_n1o"], in_=n1o[0][:])
                if stages < 5:
                    continue
                # ======== dn2 ========
                n2o = []
                for g in range(2):
                    n2o.append(bpool.tile([128, nout, 88], dt.bfloat16,
                                          tag=f"n2o{g}_{s}", name=f"n2o{g}"))
                for ocg in range(2):
                    for r0 in range(0, nout, RPP):
                        nr = min(RPP, nout - r0)
                        ps = ppool.tile([128, nr, 88], dt.float32, tag=f"ps{s}")
                        gi = 0
                        for ky in range(3):
                            for kx in range(3):
                                tap = ky * 3 + kx
                                for icc in range(2):
                                    rhs = bass.AP(
                                        n1o[icc].tensor,
                                        n1o[icc].offset + (r0 + ky) * 92 + kx + 1,
                                        [n1o[icc].ap[0], [92, nr], [1, 88]])
                                    lhs = wt["w_dn2"][:, tap * 2 + icc,
                                                      ocg * 128:(ocg + 1) * 128]
                                    nc.tensor.matmul(ps[:], lhs, rhs,
                                                     start=(gi == 0),
                                                     stop=(gi == 17))
                                    gi += 1
                        ev = wpool.tile([128, nr, 88], dt.bfloat16, tag=f"evn2{s}")
                        nc.scalar.activation(ev[:], ps[:], RELU,
                                             bias=ct["t_dn2"][:, ocg:ocg + 1],
                                             scale=ct["s_dn2"][:, ocg:ocg + 1])
                        nc.vector.tensor_copy(n2o[ocg][:, r0:r0 + nr, :], ev[:])

                if s == 0 and debug:
                    nc.sync.dma_start(out=DBG["dbg_n2o"], in_=n2o[0][:])
                if stages < 6:
                    continue
                # ======== dn3 + softmax + feat ========
                npix = nout * FW
                feat_sb[s] = bpool.tile([128, (npix + 127) // 128, CIMG],
                                        dt.bfloat16, tag=f"feat{s}", name=f"feat_sb{s}")
                depth_sb[s] = bpool.tile([128, (npix + 127) // 128, DD],
                                         dt.float32, tag=f"depth{s}", name=f"depth_sb{s}")
                n2f = [t.rearrange("p a b -> p (a b)") for t in n2o]
                for pc in range((npix + 127) // 128):
                    m = min(128, npix - pc * 128)
                    ps = ppool.tile([m, 139], dt.float32, tag=f"ps{s}")
                    for icc in range(2):
                        nc.tensor.matmul(ps[:], n2f[icc][:, pc * 128:pc * 128 + m],
                                         wt["w_dn3"][:, icc, :],
                                         start=(icc == 0), stop=(icc == 1))
                    # add bias via vector then softmax over first 59
                    lg = wpool.tile([m, 139], dt.float32, tag=f"lg{s}")
                    nc.vector.tensor_tensor(out=lg[:], in0=ps[:],
                                            in1=ct["b_dn3"][0:m],
                                            op=mybir.AluOpType.add)
                    mx = wpool.tile([m, 1], dt.float32, tag=f"mx{s}")
                    nc.vector.reduce_max(mx[:], lg[:, 0:DD],
                                         axis=mybir.AxisListType.X, negate=True)
                    ex = wpool.tile([m, DD], dt.float32, tag=f"ex{s}")
                    nc.scalar.activation(ex[:], lg[:, 0:DD],
                                         mybir.ActivationFunctionType.Exp,
                                         bias=mx[:, 0:1], scale=1.0)
                    sm = wpool.tile([m, 1], dt.float32, tag=f"sm{s}")
                    nc.vector.reduce_sum(sm[:], ex[:], axis=mybir.AxisListType.X)
                    rc = wpool.tile([m, 1], dt.float32, tag=f"rc{s}")
                    nc.vector.reciprocal(rc[:], sm[:])
                    nc.vector.tensor_scalar(out=depth_sb[s][0:m, pc, :], in0=ex[:],
                                            scalar1=rc[:, 0:1], scalar2=None,
                                            op0=mybir.AluOpType.mult)
                    nc.vector.tensor_copy(feat_sb[s][0:m, pc, :],
                                          lg[:, DD:DD + CIMG])

            # DMA outputs: global pix index = seg-A pix then seg-B pix
            for s, S in (enumerate(SEGS) if stages >= 6 else []):
                npix = S["nout"] * FW
                base = 0 if s == 0 else 16 * FW
                nfull = npix // 128
                dsl = out_depth[base:base + nfull * 128].rearrange(
                    "(a p) d -> p a d", p=128)
                nc.sync.dma_start(out=dsl, in_=depth_sb[s][:, 0:nfull, :])
                fsl = out_feat[base:base + nfull * 128].rearrange(
                    "(a p) d -> p a d", p=128)
                nc.sync.dma_start(out=fsl, in_=feat_sb[s][:, 0:nfull, :])
                rem = npix - nfull * 128
                if rem:
                    nc.sync.dma_start(
                        out=out_depth[base + nfull * 128:base + npix],
                        in_=depth_sb[s][0:rem, nfull, :])
                    nc.sync.dma_start(
                        out=out_feat[base + nfull * 128:base + npix],
                        in_=feat_sb[s][0:rem, nfull, :])
    nc.compile()
    return nc


# ------------------------------------------------------------ host helpers
def _host_geometry(rots, trans, intr, post_rots, post_trans):
    import jax
    import jax.numpy as jnp
    with jax.default_device(jax.devices("cpu")[0]):
        f32 = jnp.float32
        ds = jnp.arange(1.0, 60.0, 1.0, dtype=f32)
        xs = jnp.linspace(0.0, IW - 1.0, FW, dtype=f32)
        ys = jnp.linspace(0.0, IH - 1.0, FH, dtype=f32)
        dm = jnp.broadcast_to(ds[:, None, None], (DD, FH, FW))
        xm = jnp.broadcast_to(xs[None, None, :], (DD, FH, FW))
        ym = jnp.broadcast_to(ys[None, :, None], (DD, FH, FW))
        fr = jnp.stack([xm, ym, dm], -1)
        pts = fr[None, None] - jnp.asarray(post_trans)[:, :, None, None, None, :]
        pts = jnp.einsum("bnij,bndhwj->bndhwi",
                         jnp.linalg.inv(jnp.asarray(post_rots)), pts)
        pts = jnp.concatenate([pts[..., :2] * pts[..., 2:3], pts[..., 2:3]], -1)
        comb = jnp.einsum("bnij,bnjk->bnik", jnp.asarray(rots),
                          jnp.linalg.inv(jnp.asarray(intr)))
        pts = jnp.einsum("bnij,bndhwj->bndhwi", comb, pts) \
            + jnp.asarray(trans)[:, :, None, None, None, :]
        lo = jnp.array([XY0, XY0, Z0], dtype=f32)
        dxv = jnp.array([DXY, DXY, DZ], dtype=f32)
        g = ((pts - lo) / dxv).astype(jnp.int32).reshape(-1, 3)
        kept = ((g[:, 0] >= 0) & (g[:, 0] < NX) & (g[:, 1] >= 0) & (g[:, 1] < NX)
                & (g[:, 2] >= 0) & (g[:, 2] < NZ))
        flat = (g[:, 2] * NX + g[:, 0]) * NX + g[:, 1]
        return np.asarray(flat, np.int64), np.asarray(kept)


def _prep_a_inputs(inputs):
    """Build per-core input maps for launch A."""
    d = np.asarray(inputs["d"], np.float32).reshape(N, IH, IW)
    x_img = np.asarray(inputs["x_img"], np.float32)

    # dt1 folded affine: relu(alpha*d + beta), alpha = s*w, beta = s*b + t
    a1 = (inputs["dt1_s"] * inputs["dt1_w"][:, 0, 0, 0]).astype(np.float32)
    b1 = (inputs["dt1_s"] * inputs["dt1_b"] + inputs["dt1_t"]).astype(np.float32)
    cab = np.arange(128)
    dt1_alpha = a1[cab // 16][:, None]
    dt1_beta = b1[cab // 16][:, None]

    def wprep_dt2():
        w = np.asarray(inputs["dt2_w"], np.float32)      # [32,8,5,5]
        out = np.zeros((4, 128, 32), np.float32)
        for ky in range(5):
            for kx in range(5):
                a, dky = ky % 4, ky // 4
                bph, dmx = (kx + 2) % 4, (kx + 2) // 4
                g = dky * 2 + dmx
                rows = (np.arange(8)) * 16 + a * 4 + bph
                out[g, rows, :] = w[:, :, ky, kx].T
        return out.astype(bf16)

    def wprep_dt3():
        w = np.asarray(inputs["dt3_w"], np.float32)      # [64,32,5,5]
        out = np.zeros((9, 128, 64), np.float32)
        for ky in range(5):
            for kx in range(5):
                a, dky = ky % 2, ky // 2
                bph, dmx = kx % 2, (kx + 2) // 2 - 1
                g = dky * 3 + dmx
                rows = (a * 2 + bph) * 32 + np.arange(32)
                out[g, rows, :] = w[:, :, ky, kx].T
        return out.astype(bf16)

    def wprep_3x3(w, icc_sizes):
        O, I = w.shape[0], w.shape[1]
        nic = len(icc_sizes)
        out = np.zeros((9, nic, 128, O), np.float32)
        for ky in range(3):
            for kx in range(3):
                tap = ky * 3 + kx
                ic0 = 0
                for icc, sz in enumerate(icc_sizes):
                    out[tap, icc, 0:sz, :] = w[:, ic0:ic0 + sz, ky, kx].T
                    ic0 += sz
        return out.astype(bf16)

    # NOTE: dn1 input concat order is [dt3(64) | x_img(256)] in the reference;
    # our matmul chunks are (x0:128, x1:128, dt3:64) -> weight cols must match:
    w_dn1_full = np.asarray(inputs["dn1_w"], np.float32)
    w_dn1 = np.zeros((9, 3, 128, 256), np.float32)
    for ky in range(3):
        for kx in range(3):
            tap = ky * 3 + kx
            w_dn1[tap, 0, :, :] = w_dn1_full[:, 64:192, ky, kx].T
            w_dn1[tap, 1, :, :] = w_dn1_full[:, 192:320, ky, kx].T
            w_dn1[tap, 2, 0:64, :] = w_dn1_full[:, 0:64, ky, kx].T
    w_dn1 = w_dn1.astype(bf16)
    w_dn2 = wprep_3x3(np.asarray(inputs["dn2_w"], np.float32), [128, 128])
    w_dn3 = np.asarray(inputs["dn3_w"], np.float32)[:, :, 0, 0]  # [139, 256]
    w_dn3p = np.zeros((2, 128, 139), np.float32)
    w_dn3p[0] = w_dn3[:, 0:128].T
    w_dn3p[1] = w_dn3[:, 128:256].T

    def fold_bias(b, s, t):
        # conv bias b then bn scale/shift: relu(s*(x+b) + t) = relu(s*x + (s*b+t))
        return np.asarray(s, np.float32), np.asarray(s * b + t, np.float32)

    s2, t2 = fold_bias(inputs["dt2_b"], inputs["dt2_s"], inputs["dt2_t"])
    s3, t3 = fold_bias(inputs["dt3_b"], inputs["dt3_s"], inputs["dt3_t"])
    sn1, tn1 = fold_bias(inputs["dn1_b"], inputs["dn1_s"], inputs["dn1_t"])
    sn2, tn2 = fold_bias(inputs["dn2_b"], inputs["dn2_s"], inputs["dn2_t"])
    b_dn3 = np.broadcast_to(np.asarray(inputs["dn3_b"], np.float32)[None, :],
                            (128, 139)).copy()

    consts = np.zeros((128, 153), np.float32)
    consts[:, 0] = dt1_alpha[:, 0]
    consts[:, 1] = dt1_beta[:, 0]
    consts[:, 2] = np.tile(s2, 4)
    consts[:, 3] = np.tile(t2, 4)
    consts[:, 4] = np.tile(s3, 2)
    consts[:, 5] = np.tile(t3, 2)
    consts[:, 6:8] = sn1.reshape(2, 128).T
    consts[:, 8:10] = tn1.reshape(2, 128).T
    consts[:, 10:12] = sn2.reshape(2, 128).T
    consts[:, 12:14] = tn2.reshape(2, 128).T
    consts[:, 14:153] = b_dn3
    shared = dict(
        consts=consts,
        w_dt2=wprep_dt2(), w_dt3=wprep_dt3(), w_dn1=w_dn1, w_dn2=w_dn2,
        w_dn3=w_dn3p.astype(bf16),
    )

    maps = []
    for c in range(NCORES):
        m = dict(shared)
        for s, (cam, h0) in enumerate([SEG_A[c], SEG_B[c]]):
            S = SEGS[s]
            d0 = 8 * h0 - 34
            dseg = np.zeros((S["nd"], 712), np.float32)
            lo, hi = max(0, d0), min(IH, d0 + S["nd"])
            if hi > lo:
                dseg[lo - d0:hi - d0, 4:708] = d[cam, lo:hi]
            nq = S["nq"]
            ph = dseg.reshape(nq, 4, 178, 4)[:, :, :177, :]     # ry a rx b
            ph = ph.transpose(1, 3, 0, 2)                        # a b ry rx
            m[f"dph{s}"] = np.broadcast_to(
                ph[None], (8, 4, 4, nq, 177)).reshape(128, nq, 177).astype(bf16)
            # dmask: partition (c8,a,b) x ry -> valid(4ry+a)
            ry = np.arange(S["nd"] // 4)
            rows = 4 * ry[None, :] + (cab[:, None] // 4) % 4
            dmask = (((rows + d0) >= 0) & ((rows + d0) < IH))
            q0, t0, r0 = 2 * h0 - 8, h0 - 3, h0 - 1
            qr = np.arange(S["nt2"]) + q0
            m2m = np.broadcast_to(((qr >= 0) & (qr < 64))[None, :],
                                  (128, S["nt2"]))
            tr = np.arange(S["nt3"]) + t0
            m3m = np.broadcast_to(((tr >= 0) & (tr < FH))[None, :],
                                  (128, S["nt3"]))
            rr = np.arange(S["nn1"]) + r0
            mn1m = np.broadcast_to(((rr >= 0) & (rr < FH))[None, :],
                                   (128, S["nn1"]))
            m[f"masks{s}"] = np.concatenate(
                [dmask, m2m, m3m, mn1m], axis=1).astype(bf16)
            xseg = np.zeros((CIN, S["nt3"], FW), np.float32)
            lo2, hi2 = max(0, t0), min(FH, t0 + S["nt3"])
            if hi2 > lo2:
                xseg[:, lo2 - t0:hi2 - t0, :] = x_img[cam, :, lo2:hi2, :]
            m[f"xseg{s}"] = xseg.astype(bf16)
        maps.append(m)
    return maps


# ---------------------------------------------------------------- launch B
SPAN = 32                   # voxel span per window (one-hot width)


def build_launch_b(W):
    """W windows x QV chunks of 128 points; scatter-sum into span-32 windows.

    4 windows share one PSUM bank (partition quarters via tile_position);
    one-hot rows are built 16 chunks per DVE op; depth weights are folded
    into pb on the host."""
    nc = bacc.Bacc("TRN2", target_bir_lowering=False, debug=False,
                   num_devices=NCORES)
    NCH = W * QV                # chunk count (multiple of 32)
    NW4 = W // 4                # psum bank-groups
    pb = nc.dram_tensor("pb", [128, NCH, CIMG], dt.bfloat16,
                        kind="ExternalInput").ap()
    offv = nc.dram_tensor("offv", [128, NCH], dt.bfloat16,
                          kind="ExternalInput").ap()
    iotain = nc.dram_tensor("iotain", [128, SPAN], dt.bfloat16,
                            kind="ExternalInput").ap()
    povirt = nc.dram_tensor("povirt", [128, NW4, CIMG], dt.bfloat16,
                            kind="ExternalOutput").ap()
    BQ = 32                     # chunks per input DMA batch
    OB = 8                      # bank-groups per output DMA batch
    with tile.TileContext(nc) as tc:
        with tc.tile_pool(name="const", bufs=1) as cpool, \
             tc.tile_pool(name="io", bufs=4) as iop, \
             tc.tile_pool(name="g", bufs=4) as gp, \
             tc.tile_pool(name="ps", bufs=4, space="PSUM") as pp:
            iota = cpool.tile([128, SPAN], dt.bfloat16, name="iota")
            nc.sync.dma_start(out=iota[:], in_=iotain)
            offt = cpool.tile([128, NCH], dt.bfloat16, name="offt")
            nc.sync.dma_start(out=offt[:], in_=offv)
            ps = None
            ot = None
            for w in range(W):
                k0 = w * QV
                qr = w % 4
                if k0 % BQ == 0:
                    pbt = iop.tile([128, BQ, CIMG], dt.bfloat16,
                                   tag="pbt", name="pbt")
                    nb = min(BQ, NCH - k0)
                    nc.sync.dma_start(out=pbt[:, 0:nb, :],
                                      in_=pb[:, k0:k0 + nb, :])
                if k0 % 16 == 0:
                    # one-hot rows for 16 chunks in one batched DVE op
                    g16 = gp.tile([128, 16, SPAN], dt.bfloat16, tag="g",
                                  name="g16")
                    ia = bass.AP(iota.tensor, iota.offset,
                                 [iota.ap[0], [0, 16], [1, SPAN]])
                    ob = bass.AP(offt.tensor, offt.offset + k0,
                                 [offt.ap[0], [1, 16], [0, SPAN]])
                    nc.vector.tensor_tensor(out=g16[:], in0=ia, in1=ob,
                                            op=mybir.AluOpType.is_equal)
                if qr == 0:
                    ps = pp.tile([128, CIMG], dt.float32, tag="ps", name="ps")
                for q in range(QV):
                    k = k0 + q
                    nc.tensor.matmul(ps[qr * 32:(qr + 1) * 32, :],
                                     g16[:, k % 16, :], pbt[:, k % BQ, :],
                                     start=(q == 0), stop=(q == QV - 1),
                                     tile_position=(0, qr * 32))
                if qr == 3:
                    b = w // 4
                    if b % OB == 0:
                        ot = iop.tile([128, OB, CIMG], dt.bfloat16,
                                      tag="ot", name="ot")
                    nc.scalar.activation(ot[:, b % OB, :], ps[:],
                                         mybir.ActivationFunctionType.Copy)
                    if b % OB == OB - 1 or b == NW4 - 1:
                        b0 = (b // OB) * OB
                        nc.sync.dma_start(out=povirt[:, b0:b + 1, :],
                                          in_=ot[:, 0:b + 1 - b0, :])
    nc.compile()
    return nc


# ---------------------------------------------------------------- launch C
C_OUT_ROWS = 23              # ds2-out rows per core (8*23 = 184 >= 180)


def build_launch_c():
    nc = bacc.Bacc("TRN2", target_bir_lowering=False, debug=False,
                   num_devices=NCORES)
    NR1 = C_OUT_ROWS + 2                         # ds1-out rows incl halo (25)
    NRP = 2 * NR1 + 1                            # pooled rows needed (51)
    slab = nc.dram_tensor("slab", [CIMG, NRP, 362], dt.bfloat16,
                          kind="ExternalInput").ap()
    m1 = nc.dram_tensor("m1", [128, NR1], dt.bfloat16, kind="ExternalInput").ap()
    wd1 = nc.dram_tensor("wd1", [9, CIMG, CIMG], dt.bfloat16,
                         kind="ExternalInput").ap()
    wd2 = nc.dram_tensor("wd2", [9, CIMG, CIMG], dt.bfloat16,
                         kind="ExternalInput").ap()
    sb1 = nc.dram_tensor("sb1", [CIMG, 2], dt.float32, kind="ExternalInput").ap()
    sb2 = nc.dram_tensor("sb2", [CIMG, 2], dt.float32, kind="ExternalInput").ap()
    yout = nc.dram_tensor("yout", [CIMG, C_OUT_ROWS, 180], dt.float32,
                          kind="ExternalOutput").ap()
    RELU = mybir.ActivationFunctionType.Relu
    with tile.TileContext(nc) as tc:
        with tc.tile_pool(name="const", bufs=1) as cpool,              tc.tile_pool(name="work", bufs=2) as wp,              tc.tile_pool(name="big", bufs=1) as bp,              tc.tile_pool(name="ps", bufs=3, space="PSUM") as pp:
            slabt = bp.tile([CIMG, NRP, 362], dt.bfloat16, name="slabt")
            for rr in range(0, NRP, 13):
                nrr = min(13, NRP - rr)
                nc.sync.dma_start(out=slabt[:, rr:rr + nrr, :],
                                  in_=slab[:, rr:rr + nrr, :])
            w1 = cpool.tile([CIMG, 9, CIMG], dt.bfloat16, name="w1")
            nc.sync.dma_start(out=w1[:], in_=wd1.rearrange("t p o -> p t o"))
            w2 = cpool.tile([CIMG, 9, CIMG], dt.bfloat16, name="w2")
            nc.sync.dma_start(out=w2[:], in_=wd2.rearrange("t p o -> p t o"))
            sb1t = cpool.tile([CIMG, 2], dt.float32, name="sb1t")
            nc.sync.dma_start(out=sb1t[:], in_=sb1)
            sb2t = cpool.tile([CIMG, 2], dt.float32, name="sb2t")
            nc.sync.dma_start(out=sb2t[:], in_=sb2)
            m1t = wp.tile([128, NR1], dt.bfloat16, name="m1t")
            nc.sync.dma_start(out=m1t[:], in_=m1)
            h1 = bp.tile([CIMG, NR1, 182], dt.bfloat16, name="h1")
            nc.vector.memset(h1[:, :, 0:1], 0.0)
            nc.vector.memset(h1[:, :, 181:182], 0.0)
            # ds1: stride-2 3x3; out row t reads slab rows 2t..2t+2 (slab row 0
            # = pooled row 2o0-3, so out row t (global o0-1+t) reads
            # 2(o0-1+t)-1..+1 - (2o0-3) = 2t..2t+2); col c reads 2c..2c+2
            RP = 2
            for t0 in range(0, NR1, RP):
                nr = min(RP, NR1 - t0)
                ps = pp.tile([CIMG, nr, 180], dt.float32, tag="ps1", name="ps")
                gi = 0
                for ky in range(3):
                    for kx in range(3):
                        rhs = bass.AP(slabt.tensor,
                                      slabt.offset + (2 * t0 + ky) * 362 + kx,
                                      [slabt.ap[0], [2 * 362, nr], [2, 180]])
                        nc.tensor.matmul(ps[:], w1[:, ky * 3 + kx, :], rhs,
                                         start=(gi == 0), stop=(gi == 8))
                        gi += 1
                ev = wp.tile([CIMG, nr, 180], dt.bfloat16, tag="ev", name="ev")
                nc.scalar.activation(ev[:], ps[:], RELU, bias=sb1t[:, 1:2],
                                     scale=sb1t[:, 0:1])
                mbb = bass.AP(m1t.tensor, m1t.offset + t0,
                              [[m1t.ap[0][0], CIMG], [1, nr], [0, 180]])
                nc.vector.tensor_tensor(out=h1[:, t0:t0 + nr, 1:181],
                                        in0=ev[:], in1=mbb,
                                        op=mybir.AluOpType.mult)
            # ds2: 3x3 pad 1: out row o reads h1 rows o..o+2, col c: c..c+2
            yo = bp.tile([CIMG, C_OUT_ROWS, 180], dt.float32, name="yo")
            for o0 in range(0, C_OUT_ROWS, RP):
                nr = min(RP, C_OUT_ROWS - o0)
                ps = pp.tile([CIMG, nr, 180], dt.float32, tag="ps2", name="ps")
                gi = 0
                for ky in range(3):
                    for kx in range(3):
                        rhs = bass.AP(h1.tensor,
                                      h1.offset + (o0 + ky) * 182 + kx,
                                      [h1.ap[0], [182, nr], [1, 180]])
                        nc.tensor.matmul(ps[:], w2[:, ky * 3 + kx, :], rhs,
                                         start=(gi == 0), stop=(gi == 8))
                        gi += 1
                nc.scalar.activation(yo[:, o0:o0 + nr, :], ps[:], RELU,
                                     bias=sb2t[:, 1:2], scale=sb2t[:, 0:1])
                nc.sync.dma_start(out=yout[:, o0:o0 + nr, :],
                                  in_=yo[:, o0:o0 + nr, :])
    nc.compile()
    return nc


_CACHE = {}


def run_launch_a(inputs):
    if "A" not in _CACHE:
        _CACHE["A"] = build_launch_a()
    nc = _CACHE["A"]
    maps = _prep_a_inputs(inputs)
    res = run_bass_kernel_spmd(nc, maps, list(range(NCORES)))
    depth = np.zeros((NPIX, DD), np.float32)
    feat = np.zeros((NPIX, CIMG), np.float32)
    for c in range(NCORES):
        r = res.results[c]
        for s, (cam, h0) in enumerate([SEG_A[c], SEG_B[c]]):
            S = SEGS[s]
            npix = S["nout"] * FW
            base = (cam * FH + h0) * FW
            off = 0 if s == 0 else 16 * FW
            depth[base:base + npix] = r["out_depth"][off:off + npix]
            feat[base:base + npix] = r["out_feat"][off:off + npix].astype(np.float32)
    return depth, feat


def _build_schedule(flat, kept):
    """Sort kept points by (core, local voxel); emit fixed-quota virtual
    windows of QV*128 points with vox-span < SPAN. Returns per-core schedule
    dicts + W (max window count, rounded to 8)."""
    pts = np.arange(NPTS)
    rem = pts % (DD * FH * FW)
    d_i = rem // (FH * FW)
    col = (pts // (DD * FH * FW)) * (FH * FW) + rem % (FH * FW)
    vox = flat
    vx = (vox // NX).astype(np.int32)

    keep_idx = np.where(kept)[0]
    cnt = np.bincount(vx[keep_idx], minlength=NX)
    order = np.argsort(-cnt, kind="stable")
    core_of_row = np.zeros(NX, np.int32)
    load = np.zeros(NCORES, np.int64)
    for r in order:
        c = int(np.argmin(load))
        core_of_row[r] = c
        load[c] += cnt[r]

    row_rank = np.zeros(NX, np.int64)
    rows_of = []
    for c in range(NCORES):
        rows = np.where(core_of_row == c)[0]
        rows_of.append(rows)
        row_rank[rows] = np.arange(len(rows))

    schedules = []
    for c in range(NCORES):
        sel = keep_idx[core_of_row[vx[keep_idx]] == c]
        vloc = row_rank[vx[sel]] * NX + (vox[sel] % NX)
        o = np.argsort(vloc, kind="stable")
        sel, vloc = sel[o], vloc[o]
        win = []                      # (start, end, base)
        i, n = 0, len(sel)
        while i < n:
            base = vloc[i]
            j = min(i + QV * 128, n)
            hi = np.searchsorted(vloc, base + SPAN, "left")
            j = min(j, hi)
            win.append((i, j, base))
            i = j
        schedules.append(dict(sel=sel, vloc=vloc, win=win, col=col[sel],
                              d_i=d_i[sel], rows=rows_of[c]))
    W = max(len(s["win"]) for s in schedules)
    W = (W + 7) // 8 * 8
    return schedules, W


def _prep_b_inputs(schedules, W, depth_rows, featflat):
    maps = []
    NCH = W * QV
    iota = np.broadcast_to(np.arange(SPAN, dtype=np.float32)[None, :],
                           (128, SPAN)).astype(bf16)
    for sch in schedules:
        pb = np.zeros((128, NCH, CIMG), bf16)
        offv = np.zeros((128, NCH), bf16)
        col, d_i, vloc = sch["col"], sch["d_i"], sch["vloc"]
        dvals = depth_rows[col, d_i]                  # f32 depth weights
        wfeat = (dvals[:, None] * featflat[col]).astype(bf16)
        for w, (i, j, base) in enumerate(sch["win"]):
            L = j - i
            nch = (L + 127) // 128
            for q in range(nch):
                lo, hi = q * 128, min((q + 1) * 128, L)
                k = w * QV + q
                pb[0:hi - lo, k] = wfeat[i + lo:i + hi]
                offv[0:hi - lo, k] = (vloc[i + lo:i + hi] - base).astype(
                    np.float32)
        maps.append(dict(pb=pb, offv=offv, iotain=iota))
    return maps


def _prep_c_inputs(inputs, pooled_t):
    """pooled_t: [CIMG, 360, 360] f32 -> per-core slabs + masks + weights."""
    NR1 = C_OUT_ROWS + 2
    NRP = 2 * NR1 + 1
    w1 = np.asarray(inputs["ds1_w"], np.float32)
    w2 = np.asarray(inputs["ds2_w"], np.float32)
    wd1 = np.stack([w1[:, :, ky, kx].T for ky in range(3) for kx in range(3)])
    wd2 = np.stack([w2[:, :, ky, kx].T for ky in range(3) for kx in range(3)])
    sb1 = np.stack([np.asarray(inputs["ds1_s"], np.float32),
                    np.asarray(inputs["ds1_t"], np.float32)], 1)
    sb2 = np.stack([np.asarray(inputs["ds2_s"], np.float32),
                    np.asarray(inputs["ds2_t"], np.float32)], 1)
    shared = dict(wd1=wd1.astype(bf16), wd2=wd2.astype(bf16), sb1=sb1, sb2=sb2)
    maps = []
    pt_bf = pooled_t.astype(bf16)
    for c in range(NCORES):
        o0g = C_OUT_ROWS * c
        p0 = 2 * o0g - 3
        slab = np.zeros((CIMG, NRP, 362), bf16)
        lo, hi = max(0, p0), min(NX, p0 + NRP)
        if hi > lo:
            slab[:, lo - p0:hi - p0, 1:361] = pt_bf[:, lo:hi, :]
        t1g = np.arange(NR1) + (o0g - 1)
        m1 = np.broadcast_to(((t1g >= 0) & (t1g < 180))[None, :],
                             (128, NR1)).astype(bf16)
        maps.append(dict(shared, slab=slab, m1=np.ascontiguousarray(m1)))
    return maps


def kernel(**inputs):
    inputs = {k: np.asarray(v) for k, v in inputs.items()}
    flat, kept = _host_geometry(inputs["cam2lidar_rots"],
                                inputs["cam2lidar_trans"], inputs["intrins"],
                                inputs["post_rots"], inputs["post_trans"])
    depth_rows, feat_rows = run_launch_a(inputs)

    schedules, W = _build_schedule(flat, kept)
    key = ("B", W)
    if key not in _CACHE:
        _CACHE[key] = build_launch_b(W)
    bmaps = _prep_b_inputs(schedules, W, depth_rows, feat_rows)
    res_b = run_bass_kernel_spmd(_CACHE[key], bmaps, list(range(NCORES)))

    pooled = np.zeros((NX * NX, CIMG), np.float32)
    for c, sch in enumerate(schedules):
        virt = res_b.results[c]["povirt"].astype(np.float32)  # [128, NW4, C]
        rows_arr = sch["rows"]
        nloc = len(rows_arr) * NX
        for w, (i, j, base) in enumerate(sch["win"]):
            span = min(SPAN, nloc - base)
            lidx = base + np.arange(span)
            ridx = rows_arr[lidx // NX] * NX + (lidx % NX)
            qr = w % 4
            pooled[ridx] += virt[qr * 32:qr * 32 + span, w // 4]
    pooled_t = np.ascontiguousarray(
        pooled.reshape(NX, NX, CIMG).transpose(2, 0, 1))

    if "C" not in _CACHE:
        _CACHE["C"] = build_launch_c()
    cmaps = _prep_c_inputs(inputs, pooled_t)
    res_c = run_bass_kernel_spmd(_CACHE["C"], cmaps, list(range(NCORES)))
    out = np.zeros((1, CIMG, 180, 180), np.float32)
    for c in range(NCORES):
        o0g = C_OUT_ROWS * c
        nr = min(C_OUT_ROWS, 180 - o0g)
        if nr > 0:
            out[0, :, o0g:o0g + nr, :] = res_c.results[c]["yout"][:, 0:nr, :]
    return out



# revision 13
# speedup vs baseline: 1.3363x; 1.0774x over previous
"""DepthLSSTransform Trainium kernel: 3 SPMD launches over 8 NeuronCores.

Launch A: per-camera conv pipeline (dtransform + depthnet + softmax) on
          24-row bands (one 16-row + one 8-row segment per core).
Launch B: bev_pool segment-sum via one-hot matmuls over a host-built
          virtual-window schedule (sorted-by-voxel points).
Launch C: BEV downsample convs, spatially sharded.
Host: geometry/voxel indices, scheduling, gathers, folds (orchestration).
"""
import numpy as np
import ml_dtypes

import concourse.bass as bass
import concourse.tile as tile
from concourse import bacc, mybir
from concourse.bass_utils import run_bass_kernel_spmd

dt = mybir.dt
bf16 = ml_dtypes.bfloat16

# ---- problem constants (hardcoded per contract) ----
B, N = 1, 6
CIN, CIMG, DD = 256, 80, 59
FH, FW, IH, IW = 32, 88, 256, 704
XY0, DXY, NX = -54.0, 0.3, 360
Z0, DZ, NZ = -10.0, 20.0, 1
NPTS = N * DD * FH * FW
NPIX = N * FH * FW
NCORES = 8
QV = 4                      # chunks of 128 points per virtual window

# per-core segments: (camera, h0) for seg A (16 rows) and seg B (8 rows)
SEG_A = [(0, 0), (1, 0), (1, 16), (2, 16), (3, 0), (4, 0), (4, 16), (5, 16)]
SEG_B = [(0, 16), (0, 24), (2, 0), (2, 8), (3, 16), (3, 24), (5, 0), (5, 8)]
# band pixel ranges in global row order (row = n*32 + h)
ROWS_OF_CORE = [[(SEG_A[c][0] * FH + SEG_A[c][1] + r) for r in range(16)] +
                [(SEG_B[c][0] * FH + SEG_B[c][1] + r) for r in range(8)]
                for c in range(NCORES)]

# segment geometry: rows16 segment: d rows [8h0-34, 8h0+158) (192), dt2 out
# rows [2h0-8, 2h0+39) (47), dt3 [h0-3, h0+19) (22), dn1 [h0-1, h0+17) (18)
SEGS = [dict(nout=16, nd=192, nq=48, nt2=47, nt3=22, nn1=18),
        dict(nout=8, nd=128, nq=32, nt2=31, nt3=14, nn1=10)]


def _seg_ranges(h0, S):
    return dict(d0=8 * h0 - 34, q0=2 * h0 - 8, t0=h0 - 3, r0=h0 - 1, o0=h0)


# ---------------------------------------------------------------- launch A
# edge-mask layout per segment: dt2 rows [0:8]+[nt2-7:nt2], dt3 rows
# [0:3]+[nt3-3:nt3], dn1 rows [0:1]+[nn1-1:nn1]  -> 23 columns
EMSK = 23


def build_launch_a(debug=False, psum_bufs=4, work_bufs=3, stages=9):
    nc = bacc.Bacc("TRN2", target_bir_lowering=False, debug=False,
                   num_devices=NCORES)
    AP = {}

    def inp(name, shape, dtype=dt.bfloat16):
        AP[name] = nc.dram_tensor(name, shape, dtype, kind="ExternalInput").ap()
        return AP[name]

    # per segment inputs (s = 0: 16-row, 1: 8-row)
    for s, S in enumerate(SEGS):
        inp(f"dph{s}", [128, S["nq"], 177])             # poison-filled OOB
        inp(f"masks{s}", [128, EMSK])                   # edge-row masks
        inp(f"xseg{s}", [CIN, S["nt3"], FW])            # x_img slice (zeroed oob)
    # packed f32 constants: [alpha, beta, s_dt2, t_dt2, s_dt3, t_dt3,
    #  s_dn1(2), t_dn1(2), s_dn2(2), t_dn2(2), b_dn3(139)] -> [128, 153]
    inp("consts", [128, 153], dt.float32)
    inp("bias3", [1, 139])                              # dn3 bias row bf16
    # conv weights (host-prepped layouts)
    inp("w_dt2", [4, 128, 32])                          # groups (dky,dmx)
    inp("w_dt3", [9, 128, 64])
    inp("w_dn1", [9, 3, 128, 256])                      # tap, icchunk(128,128,64pad) -> 256
    inp("w_dn2", [9, 2, 128, 256])
    inp("w_dn3", [2, 128, 139])

    DBG = {}
    dbg_specs = [] if not debug else [("dbg_t1", [128, SEGS[0]["nq"], 177], dt.bfloat16),
                        ("dbg_dt2o", [32, SEGS[0]["nt2"] + 1, 180], dt.bfloat16),
                        ("dbg_dtc", [64, SEGS[0]["nt3"], 92], dt.bfloat16),
                        ("dbg_n1o", [128, SEGS[0]["nn1"], 92], dt.bfloat16),
                        ("dbg_n2o", [128, SEGS[0]["nout"], 88], dt.bfloat16)]
    for nm, sh, dty in dbg_specs:
        DBG[nm] = nc.dram_tensor(nm, sh, dty, kind="ExternalOutput").ap()
    out_depth = nc.dram_tensor("out_depth", [24 * FW, DD], dt.float32,
                               kind="ExternalOutput").ap()
    out_feat = nc.dram_tensor("out_feat", [24 * FW, CIMG], dt.bfloat16,
                              kind="ExternalOutput").ap()

    # HBM scratch
    scr = {}
    for s, S in enumerate(SEGS):
        scr[f"dt2o{s}"] = nc.dram_tensor(f"dt2o{s}", [32, S["nt2"] + 1, 2, 90], dt.bfloat16).ap()

    RELU = mybir.ActivationFunctionType.Relu
    with tile.TileContext(nc) as tc:
        with tc.tile_pool(name="const", bufs=1) as cpool, \
             tc.tile_pool(name="work", bufs=work_bufs) as wpool, \
             tc.tile_pool(name="big", bufs=1) as bpool, \
             tc.tile_pool(name="psum", bufs=psum_bufs, space="PSUM") as ppool:
            # ---- load packed constants in one DMA ----
            cts = cpool.tile([128, 153], dt.float32, name="cts")
            nc.sync.dma_start(out=cts[:], in_=AP["consts"])
            ct = {"dt1_alpha": cts[:, 0:1], "dt1_beta": cts[:, 1:2],
                  "s_dt2": cts[:, 2:3], "t_dt2": cts[:, 3:4],
                  "s_dt3": cts[:, 4:5], "t_dt3": cts[:, 5:6],
                  "s_dn1": cts[:, 6:8], "t_dn1": cts[:, 8:10],
                  "s_dn2": cts[:, 10:12], "t_dn2": cts[:, 12:14],
                  "b_dn3": cts[:, 14:153]}
            wt = {}
            for nm, pat in [("w_dt2", "g p o -> p g o"),
                            ("w_dt3", "g p o -> p g o"),
                            ("w_dn1", "t i p o -> p (t i) o"),
                            ("w_dn2", "t i p o -> p (t i) o"),
                            ("w_dn3", "g p o -> p g o")]:
                sh = list(AP[nm].shape)
                wt[nm] = cpool.tile([sh[-2], int(np.prod(sh[:-2])), sh[-1]],
                                    dt.bfloat16, tag=nm, name=f'wt_{nm}')
                nc.sync.dma_start(out=wt[nm][:], in_=AP[nm].rearrange(pat))

            b3t = cpool.tile([1, 139], dt.bfloat16, name="b3t")
            nc.sync.dma_start(out=b3t[:], in_=AP["bias3"])
            ones1 = cpool.tile([1, 128], dt.bfloat16, name="ones1")
            nc.vector.memset(ones1[:], 1.0)

            feat_sb = {}
            depth_sb = {}
            for s, S in enumerate(SEGS):
                nq, nt2, nt3, nn1, nout = S["nq"], S["nt2"], S["nt3"], S["nn1"], S["nout"]
                # ======== dt1 : relu(alpha*d + beta) on ACT; OOB poisoned ====
                dph = bpool.tile([128, nq, 177], dt.bfloat16, tag=f"dph{s}")
                for qq in range(0, nq, nq // 4):
                    nqq = min(nq // 4, nq - qq)
                    nc.sync.dma_start(out=dph[:, qq:qq + nqq, :],
                                      in_=AP[f"dph{s}"][:, qq:qq + nqq, :])
                t1 = bpool.tile([128, nq, 177], dt.bfloat16, tag=f"t1{s}")
                mall = wpool.tile([128, EMSK], dt.bfloat16,
                                  tag=f"msk{s}", name="mall")
                nc.sync.dma_start(out=mall[:], in_=AP[f"masks{s}"])
                QCH = nq // 4
                for qq in range(0, nq, QCH):
                    nqq = min(QCH, nq - qq)
                    sl = (slice(None), slice(qq, qq + nqq), slice(None))
                    nc.scalar.activation(t1[sl], dph[sl], RELU,
                                         bias=ct["dt1_beta"][:, 0:1],
                                         scale=ct["dt1_alpha"][:, 0:1])
                if s == 0 and debug:
                    nc.sync.dma_start(out=DBG["dbg_t1"], in_=t1[:])

                if stages < 2:
                    continue
                # ======== dt2 ========
                o2 = bpool.tile([32, nt2 + 1, 180], dt.bfloat16, tag=f"o2{s}")
                nc.vector.memset(o2[:, :, 0:1], 0.0)
                nc.vector.memset(o2[:, :, 89:91], 0.0)
                nc.vector.memset(o2[:, :, 179:180], 0.0)
                nc.vector.memset(o2[:, nt2:nt2 + 1, :], 0.0)
                RPP2 = 2
                for q0 in range(0, nt2, RPP2):
                    nr = min(RPP2, nt2 - q0)
                    ps = ppool.tile([32, nr, 176], dt.float32, tag=f"ps{s}", name="ps2")
                    gi = 0
                    for dky in range(2):
                        for dmx in range(2):
                            g = dky * 2 + dmx
                            rhs = bass.AP(
                                t1.tensor, t1.offset + (q0 + dky) * 177 + dmx,
                                [t1.ap[0], [177, nr], [1, 176]])
                            nc.tensor.matmul(ps[:], wt["w_dt2"][:, g, :], rhs,
                                             start=(gi == 0), stop=(gi == 3))
                            gi += 1
                    # write col c at (c%2)*90 + c//2 + 1  (phase-split layout)
                    o2dst = bass.AP(o2.tensor, o2.offset + q0 * 180 + 1,
                                    [[o2.ap[0][0], 32], [180, nr],
                                     [1, 88], [90, 2]])
                    nc.scalar.activation(o2dst, ps[:], RELU,
                                         bias=ct["t_dt2"][0:32, 0:1],
                                         scale=ct["s_dt2"][0:32, 0:1])
                # zero image-OOB edge rows (masks: interior cores all-ones)
                mlo = bass.AP(mall.tensor, mall.offset,
                              [[mall.ap[0][0], 32], [1, 8], [0, 180]])
                nc.vector.tensor_tensor(out=o2[:, 0:8, :], in0=o2[:, 0:8, :],
                                        in1=mlo, op=mybir.AluOpType.mult)
                mhi = bass.AP(mall.tensor, mall.offset + 8,
                              [[mall.ap[0][0], 32], [1, 7], [0, 180]])
                nc.vector.tensor_tensor(out=o2[:, nt2 - 7:nt2, :],
                                        in0=o2[:, nt2 - 7:nt2, :],
                                        in1=mhi, op=mybir.AluOpType.mult)
                nc.sync.dma_start(out=scr[f"dt2o{s}"],
                                  in_=o2.rearrange("p q (b x) -> p q b x", b=2))
                if s == 0 and debug:
                    nc.sync.dma_start(out=DBG["dbg_dt2o"], in_=o2[:])

                if stages < 3:
                    continue
                # ======== dt3 ========
                nry3 = nt3 + 2
                ph3 = bpool.tile([128, nry3, 90], dt.bfloat16, tag=f"ph3{s}")
                sd2 = scr[f"dt2o{s}"]
                for a2 in range(2):
                    for b2 in range(2):
                        pap3 = bass.AP(sd2.tensor,
                                       sd2.offset + a2 * 180 + b2 * 90,
                                       [[(nt2 + 1) * 180, 32],
                                        [2 * 180, nry3], [1, 90]])
                        nc.sync.dma_start(
                            out=ph3[(a2 * 2 + b2) * 32:(a2 * 2 + b2 + 1) * 32],
                            in_=pap3)
                # concat input tile: [64 dt3 | pad] plus x_img tiles
                dtc = bpool.tile([64, nt3, 92], dt.bfloat16, tag=f"dtc{s}")
                nc.vector.memset(dtc[:, :, 0:2], 0.0)
                nc.vector.memset(dtc[:, :, 90:92], 0.0)
                RPP3 = 4
                for t0 in range(0, nt3, RPP3):
                    nr = min(RPP3, nt3 - t0)
                    ps = ppool.tile([64, nr, 88], dt.float32, tag=f"ps{s}")
                    gi = 0
                    for dky in range(3):
                        for dmx in range(3):
                            g = dky * 3 + dmx
                            rhs = bass.AP(ph3.tensor,
                                          ph3.offset + (t0 + dky) * 90 + dmx,
                                          [ph3.ap[0], [90, nr], [1, 88]])
                            nc.tensor.matmul(ps[:], wt["w_dt3"][:, g, :], rhs,
                                             start=(gi == 0), stop=(gi == 8))
                            gi += 1
                    nc.scalar.activation(dtc[:, t0:t0 + nr, 2:90], ps[:], RELU,
                                         bias=ct["t_dt3"][0:64, 0:1],
                                         scale=ct["s_dt3"][0:64, 0:1])
                m3lo = bass.AP(mall.tensor, mall.offset + 15,
                               [[mall.ap[0][0], 64], [1, 3], [0, 92]])
                nc.vector.tensor_tensor(out=dtc[:, 0:3, :], in0=dtc[:, 0:3, :],
                                        in1=m3lo, op=mybir.AluOpType.mult)
                m3hi = bass.AP(mall.tensor, mall.offset + 18,
                               [[mall.ap[0][0], 64], [1, 3], [0, 92]])
                nc.vector.tensor_tensor(out=dtc[:, nt3 - 3:nt3, :],
                                        in0=dtc[:, nt3 - 3:nt3, :],
                                        in1=m3hi, op=mybir.AluOpType.mult)

                if s == 0 and debug:
                    nc.sync.dma_start(out=DBG["dbg_dtc"], in_=dtc[:])
                if stages < 4:
                    continue
                # ======== dn1 ========
                xs = []
                for g in range(2):
                    xt = bpool.tile([128, nt3, 92], dt.bfloat16, tag=f"x{g}_{s}",
                                     name=f"xseg_t{g}")
                    nc.vector.memset(xt[:, :, 0:2], 0.0)
                    nc.vector.memset(xt[:, :, 90:92], 0.0)
                    nc.sync.dma_start(
                        out=xt[:, :, 2:90],
                        in_=AP[f"xseg{s}"][g * 128:(g + 1) * 128])
                    xs.append(xt)
                n1o = []
                for g in range(2):
                    t = bpool.tile([128, nn1, 92], dt.bfloat16, tag=f"n1o{g}_{s}")
                    nc.vector.memset(t[:, :, 0:2], 0.0)
                    nc.vector.memset(t[:, :, 90:92], 0.0)
                    n1o.append(t)
                RPP = 5
                for ocg in range(2):
                    for r0 in range(0, nn1, RPP):
                        nr = min(RPP, nn1 - r0)
                        ps = ppool.tile([128, nr, 88], dt.float32, tag=f"ps{s}")
                        gi = 0
                        for ky in range(3):
                            for kx in range(3):
                                tap = ky * 3 + kx
                                for icc, srcT in enumerate((xs[0], xs[1], dtc)):
                                    kk = 128 if icc < 2 else 64
                                    rhs = bass.AP(
                                        srcT.tensor,
                                        srcT.offset + (r0 + ky + 1) * 92 + kx + 1,
                                        [srcT.ap[0], [92, nr], [1, 88]])
                                    lhs = wt["w_dn1"][0:kk, tap * 3 + icc,
                                                      ocg * 128:(ocg + 1) * 128]
                                    nc.tensor.matmul(ps[:], lhs, rhs,
                                                     start=(gi == 0),
                                                     stop=(gi == 26))
                                    gi += 1
                        nc.scalar.activation(n1o[ocg][:, r0:r0 + nr, 2:90],
                                             ps[:], RELU,
                                             bias=ct["t_dn1"][:, ocg:ocg + 1],
                                             scale=ct["s_dn1"][:, ocg:ocg + 1])
                for g in range(2):
                    mnlo = bass.AP(mall.tensor, mall.offset + 21,
                                   [mall.ap[0], [1, 1], [0, 92]])
                    nc.vector.tensor_tensor(out=n1o[g][:, 0:1, :],
                                            in0=n1o[g][:, 0:1, :],
                                            in1=mnlo, op=mybir.AluOpType.mult)
                    mnhi = bass.AP(mall.tensor, mall.offset + 22,
                                   [mall.ap[0], [1, 1], [0, 92]])
                    nc.vector.tensor_tensor(out=n1o[g][:, nn1 - 1:nn1, :],
                                            in0=n1o[g][:, nn1 - 1:nn1, :],
                                            in1=mnhi, op=mybir.AluOpType.mult)

                if s == 0 and debug:
                    nc.sync.dma_start(out=DBG["dbg_n1o"], in_=n1o[0][:])
                if stages < 5:
                    continue
                # ======== dn2 ========
                n2o = []
                for g in range(2):
                    n2o.append(bpool.tile([128, nout, 88], dt.bfloat16,
                                          tag=f"n2o{g}_{s}", name=f"n2o{g}"))
                for ocg in range(2):
                    for r0 in range(0, nout, RPP):
                        nr = min(RPP, nout - r0)
                        ps = ppool.tile([128, nr, 88], dt.float32, tag=f"ps{s}")
                        gi = 0
                        for ky in range(3):
                            for kx in range(3):
                                tap = ky * 3 + kx
                                for icc in range(2):
                                    rhs = bass.AP(
                                        n1o[icc].tensor,
                                        n1o[icc].offset + (r0 + ky) * 92 + kx + 1,
                                        [n1o[icc].ap[0], [92, nr], [1, 88]])
                                    lhs = wt["w_dn2"][:, tap * 2 + icc,
                                                      ocg * 128:(ocg + 1) * 128]
                                    nc.tensor.matmul(ps[:], lhs, rhs,
                                                     start=(gi == 0),
                                                     stop=(gi == 17))
                                    gi += 1
                        nc.scalar.activation(n2o[ocg][:, r0:r0 + nr, :], ps[:],
                                             RELU,
                                             bias=ct["t_dn2"][:, ocg:ocg + 1],
                                             scale=ct["s_dn2"][:, ocg:ocg + 1])

                if s == 0 and debug:
                    nc.sync.dma_start(out=DBG["dbg_n2o"], in_=n2o[0][:])
                if stages < 6:
                    continue
                # ======== dn3 + softmax + feat ========
                npix = nout * FW
                feat_sb[s] = bpool.tile([128, (npix + 127) // 128, CIMG],
                                        dt.bfloat16, tag=f"feat{s}", name=f"feat_sb{s}")
                depth_sb[s] = bpool.tile([128, (npix + 127) // 128, DD],
                                         dt.float32, tag=f"depth{s}", name=f"depth_sb{s}")
                n2f = [t.rearrange("p a b -> p (a b)") for t in n2o]
                for pc in range((npix + 127) // 128):
                    m = min(128, npix - pc * 128)
                    ps = ppool.tile([m, 139], dt.float32, tag=f"ps{s}")
                    for icc in range(2):
                        nc.tensor.matmul(ps[:], n2f[icc][:, pc * 128:pc * 128 + m],
                                         wt["w_dn3"][:, icc, :],
                                         start=(icc == 0), stop=False)
                    # bias row via rank-1 matmul: ones[1,m].T @ b3t[1,139]
                    nc.tensor.matmul(ps[:], ones1[0:1, 0:m], b3t[:],
                                     start=False, stop=True)
                    # softmax over first 59 channels, straight from PSUM
                    mx = wpool.tile([m, 1], dt.float32, tag=f"mx{s}")
                    nc.vector.reduce_max(mx[:], ps[:, 0:DD],
                                         axis=mybir.AxisListType.X, negate=True)
                    ex = wpool.tile([m, DD], dt.float32, tag=f"ex{s}")
                    nc.scalar.activation(ex[:], ps[:, 0:DD],
                                         mybir.ActivationFunctionType.Exp,
                                         bias=mx[:, 0:1], scale=1.0)
                    sm = wpool.tile([m, 1], dt.float32, tag=f"sm{s}")
                    nc.vector.reduce_sum(sm[:], ex[:], axis=mybir.AxisListType.X)
                    rc = wpool.tile([m, 1], dt.float32, tag=f"rc{s}")
                    nc.vector.reciprocal(rc[:], sm[:])
                    nc.vector.tensor_scalar(out=depth_sb[s][0:m, pc, :], in0=ex[:],
                                            scalar1=rc[:, 0:1], scalar2=None,
                                            op0=mybir.AluOpType.mult)
                    nc.vector.tensor_copy(feat_sb[s][0:m, pc, :],
                                          ps[:, DD:DD + CIMG])

            # DMA outputs: global pix index = seg-A pix then seg-B pix
            for s, S in (enumerate(SEGS) if stages >= 6 else []):
                npix = S["nout"] * FW
                base = 0 if s == 0 else 16 * FW
                nfull = npix // 128
                dsl = out_depth[base:base + nfull * 128].rearrange(
                    "(a p) d -> p a d", p=128)
                nc.sync.dma_start(out=dsl, in_=depth_sb[s][:, 0:nfull, :])
                fsl = out_feat[base:base + nfull * 128].rearrange(
                    "(a p) d -> p a d", p=128)
                nc.sync.dma_start(out=fsl, in_=feat_sb[s][:, 0:nfull, :])
                rem = npix - nfull * 128
                if rem:
                    nc.sync.dma_start(
                        out=out_depth[base + nfull * 128:base + npix],
                        in_=depth_sb[s][0:rem, nfull, :])
                    nc.sync.dma_start(
                        out=out_feat[base + nfull * 128:base + npix],
                        in_=feat_sb[s][0:rem, nfull, :])
    nc.compile()
    return nc


# ------------------------------------------------------------ host helpers
def _host_geometry(rots, trans, intr, post_rots, post_trans):
    import jax
    import jax.numpy as jnp
    with jax.default_device(jax.devices("cpu")[0]):
        f32 = jnp.float32
        ds = jnp.arange(1.0, 60.0, 1.0, dtype=f32)
        xs = jnp.linspace(0.0, IW - 1.0, FW, dtype=f32)
        ys = jnp.linspace(0.0, IH - 1.0, FH, dtype=f32)
        dm = jnp.broadcast_to(ds[:, None, None], (DD, FH, FW))
        xm = jnp.broadcast_to(xs[None, None, :], (DD, FH, FW))
        ym = jnp.broadcast_to(ys[None, :, None], (DD, FH, FW))
        fr = jnp.stack([xm, ym, dm], -1)
        pts = fr[None, None] - jnp.asarray(post_trans)[:, :, None, None, None, :]
        pts = jnp.einsum("bnij,bndhwj->bndhwi",
                         jnp.linalg.inv(jnp.asarray(post_rots)), pts)
        pts = jnp.concatenate([pts[..., :2] * pts[..., 2:3], pts[..., 2:3]], -1)
        comb = jnp.einsum("bnij,bnjk->bnik", jnp.asarray(rots),
                          jnp.linalg.inv(jnp.asarray(intr)))
        pts = jnp.einsum("bnij,bndhwj->bndhwi", comb, pts) \
            + jnp.asarray(trans)[:, :, None, None, None, :]
        lo = jnp.array([XY0, XY0, Z0], dtype=f32)
        dxv = jnp.array([DXY, DXY, DZ], dtype=f32)
        g = ((pts - lo) / dxv).astype(jnp.int32).reshape(-1, 3)
        kept = ((g[:, 0] >= 0) & (g[:, 0] < NX) & (g[:, 1] >= 0) & (g[:, 1] < NX)
                & (g[:, 2] >= 0) & (g[:, 2] < NZ))
        flat = (g[:, 2] * NX + g[:, 0]) * NX + g[:, 1]
        return np.asarray(flat, np.int64), np.asarray(kept)


def _prep_a_inputs(inputs):
    """Build per-core input maps for launch A."""
    d = np.asarray(inputs["d"], np.float32).reshape(N, IH, IW)
    x_img = np.asarray(inputs["x_img"], np.float32)

    # dt1 folded affine: relu(alpha*d + beta), alpha = s*w, beta = s*b + t
    a1 = (inputs["dt1_s"] * inputs["dt1_w"][:, 0, 0, 0]).astype(np.float32)
    b1 = (inputs["dt1_s"] * inputs["dt1_b"] + inputs["dt1_t"]).astype(np.float32)
    cab = np.arange(128)
    dt1_alpha = a1[cab // 16][:, None]
    dt1_beta = b1[cab // 16][:, None]

    def wprep_dt2():
        w = np.asarray(inputs["dt2_w"], np.float32)      # [32,8,5,5]
        out = np.zeros((4, 128, 32), np.float32)
        for ky in range(5):
            for kx in range(5):
                a, dky = ky % 4, ky // 4
                bph, dmx = (kx + 2) % 4, (kx + 2) // 4
                g = dky * 2 + dmx
                rows = (np.arange(8)) * 16 + a * 4 + bph
                out[g, rows, :] = w[:, :, ky, kx].T
        return out.astype(bf16)

    def wprep_dt3():
        w = np.asarray(inputs["dt3_w"], np.float32)      # [64,32,5,5]
        out = np.zeros((9, 128, 64), np.float32)
        for ky in range(5):
            for kx in range(5):
                a, dky = ky % 2, ky // 2
                bph, dmx = kx % 2, (kx + 2) // 2 - 1
                g = dky * 3 + dmx
                rows = (a * 2 + bph) * 32 + np.arange(32)
                out[g, rows, :] = w[:, :, ky, kx].T
        return out.astype(bf16)

    def wprep_3x3(w, icc_sizes):
        O, I = w.shape[0], w.shape[1]
        nic = len(icc_sizes)
        out = np.zeros((9, nic, 128, O), np.float32)
        for ky in range(3):
            for kx in range(3):
                tap = ky * 3 + kx
                ic0 = 0
                for icc, sz in enumerate(icc_sizes):
                    out[tap, icc, 0:sz, :] = w[:, ic0:ic0 + sz, ky, kx].T
                    ic0 += sz
        return out.astype(bf16)

    # NOTE: dn1 input concat order is [dt3(64) | x_img(256)] in the reference;
    # our matmul chunks are (x0:128, x1:128, dt3:64) -> weight cols must match:
    w_dn1_full = np.asarray(inputs["dn1_w"], np.float32)
    w_dn1 = np.zeros((9, 3, 128, 256), np.float32)
    for ky in range(3):
        for kx in range(3):
            tap = ky * 3 + kx
            w_dn1[tap, 0, :, :] = w_dn1_full[:, 64:192, ky, kx].T
            w_dn1[tap, 1, :, :] = w_dn1_full[:, 192:320, ky, kx].T
            w_dn1[tap, 2, 0:64, :] = w_dn1_full[:, 0:64, ky, kx].T
    w_dn1 = w_dn1.astype(bf16)
    w_dn2 = wprep_3x3(np.asarray(inputs["dn2_w"], np.float32), [128, 128])
    w_dn3 = np.asarray(inputs["dn3_w"], np.float32)[:, :, 0, 0]  # [139, 256]
    w_dn3p = np.zeros((2, 128, 139), np.float32)
    w_dn3p[0] = w_dn3[:, 0:128].T
    w_dn3p[1] = w_dn3[:, 128:256].T

    def fold_bias(b, s, t):
        # conv bias b then bn scale/shift: relu(s*(x+b) + t) = relu(s*x + (s*b+t))
        return np.asarray(s, np.float32), np.asarray(s * b + t, np.float32)

    s2, t2 = fold_bias(inputs["dt2_b"], inputs["dt2_s"], inputs["dt2_t"])
    s3, t3 = fold_bias(inputs["dt3_b"], inputs["dt3_s"], inputs["dt3_t"])
    sn1, tn1 = fold_bias(inputs["dn1_b"], inputs["dn1_s"], inputs["dn1_t"])
    sn2, tn2 = fold_bias(inputs["dn2_b"], inputs["dn2_s"], inputs["dn2_t"])
    b_dn3 = np.broadcast_to(np.asarray(inputs["dn3_b"], np.float32)[None, :],
                            (128, 139)).copy()

    consts = np.zeros((128, 153), np.float32)
    consts[:, 0] = dt1_alpha[:, 0]
    consts[:, 1] = dt1_beta[:, 0]
    consts[:, 2] = np.tile(s2, 4)
    consts[:, 3] = np.tile(t2, 4)
    consts[:, 4] = np.tile(s3, 2)
    consts[:, 5] = np.tile(t3, 2)
    consts[:, 6:8] = sn1.reshape(2, 128).T
    consts[:, 8:10] = tn1.reshape(2, 128).T
    consts[:, 10:12] = sn2.reshape(2, 128).T
    consts[:, 12:14] = tn2.reshape(2, 128).T
    consts[:, 14:153] = b_dn3
    shared = dict(
        consts=consts,
        w_dt2=wprep_dt2(), w_dt3=wprep_dt3(), w_dn1=w_dn1, w_dn2=w_dn2,
        w_dn3=w_dn3p.astype(bf16),
        bias3=np.asarray(inputs["dn3_b"], np.float32)[None, :].astype(bf16),
    )
    # poison for OOB dt1 inputs: relu(alpha*poison + beta) == 0
    pois = np.where(a1 >= 0, -1e30, 1e30).astype(np.float32)

    maps = []
    for c in range(NCORES):
        m = dict(shared)
        for s, (cam, h0) in enumerate([SEG_A[c], SEG_B[c]]):
            S = SEGS[s]
            d0 = 8 * h0 - 34
            dseg = np.full((S["nd"], 712), np.inf, np.float32)
            lo, hi = max(0, d0), min(IH, d0 + S["nd"])
            if hi > lo:
                dseg[lo - d0:hi - d0, 4:708] = d[cam, lo:hi]
            nq = S["nq"]
            ph = dseg.reshape(nq, 4, 178, 4)[:, :, :177, :]     # ry a rx b
            ph = ph.transpose(1, 3, 0, 2)                        # a b ry rx
            dphf = np.broadcast_to(ph[None], (8, 4, 4, nq, 177)).copy()
            dphf = np.where(np.isfinite(dphf), dphf,
                            pois[:, None, None, None, None])
            m[f"dph{s}"] = dphf.reshape(128, nq, 177).astype(bf16)
            # edge-row masks (image-boundary zeroing after each conv)
            q0, t0, r0 = 2 * h0 - 8, h0 - 3, h0 - 1
            q_lo = q0 + np.arange(8)
            q_hi = q0 + (S["nt2"] - 7) + np.arange(7)
            t_lo = t0 + np.arange(3)
            t_hi = t0 + (S["nt3"] - 3) + np.arange(3)
            r_lo = r0 + np.arange(1)
            r_hi = r0 + (S["nn1"] - 1) + np.arange(1)
            em = np.concatenate([
                (q_lo >= 0) & (q_lo < 64), (q_hi >= 0) & (q_hi < 64),
                (t_lo >= 0) & (t_lo < FH), (t_hi >= 0) & (t_hi < FH),
                (r_lo >= 0) & (r_lo < FH), (r_hi >= 0) & (r_hi < FH)])
            m[f"masks{s}"] = np.broadcast_to(
                em[None, :], (128, EMSK)).astype(bf16)
            xseg = np.zeros((CIN, S["nt3"], FW), np.float32)
            lo2, hi2 = max(0, t0), min(FH, t0 + S["nt3"])
            if hi2 > lo2:
                xseg[:, lo2 - t0:hi2 - t0, :] = x_img[cam, :, lo2:hi2, :]
            m[f"xseg{s}"] = xseg.astype(bf16)
        maps.append(m)
    return maps


# ---------------------------------------------------------------- launch B
SPAN = 32                   # voxel span per window (one-hot width)


def build_launch_b(W):
    """W windows x QV chunks of 128 points; scatter-sum into span-32 windows.

    4 windows share one PSUM bank (partition quarters via tile_position);
    one-hot rows are built 16 chunks per DVE op; depth weights are folded
    into pb on the host."""
    nc = bacc.Bacc("TRN2", target_bir_lowering=False, debug=False,
                   num_devices=NCORES)
    NCH = W * QV                # chunk count (multiple of 32)
    NW4 = W // 4                # psum bank-groups
    pb = nc.dram_tensor("pb", [128, NCH, CIMG], dt.bfloat16,
                        kind="ExternalInput").ap()
    offv = nc.dram_tensor("offv", [128, NCH], dt.bfloat16,
                          kind="ExternalInput").ap()
    iotain = nc.dram_tensor("iotain", [128, SPAN], dt.bfloat16,
                            kind="ExternalInput").ap()
    povirt = nc.dram_tensor("povirt", [128, NW4, CIMG], dt.bfloat16,
                            kind="ExternalOutput").ap()
    BQ = 32                     # chunks per input DMA batch
    OB = 8                      # bank-groups per output DMA batch
    with tile.TileContext(nc) as tc:
        with tc.tile_pool(name="const", bufs=1) as cpool, \
             tc.tile_pool(name="io", bufs=4) as iop, \
             tc.tile_pool(name="g", bufs=4) as gp, \
             tc.tile_pool(name="ps", bufs=4, space="PSUM") as pp:
            iota = cpool.tile([128, SPAN], dt.bfloat16, name="iota")
            nc.sync.dma_start(out=iota[:], in_=iotain)
            offt = cpool.tile([128, NCH], dt.bfloat16, name="offt")
            nc.sync.dma_start(out=offt[:], in_=offv)
            ps = None
            ot = None
            for w in range(W):
                k0 = w * QV
                qr = w % 4
                if k0 % BQ == 0:
                    pbt = iop.tile([128, BQ, CIMG], dt.bfloat16,
                                   tag="pbt", name="pbt")
                    nb = min(BQ, NCH - k0)
                    nc.sync.dma_start(out=pbt[:, 0:nb, :],
                                      in_=pb[:, k0:k0 + nb, :])
                if k0 % 16 == 0:
                    # one-hot rows for 16 chunks in one batched DVE op
                    g16 = gp.tile([128, 16, SPAN], dt.bfloat16, tag="g",
                                  name="g16")
                    ia = bass.AP(iota.tensor, iota.offset,
                                 [iota.ap[0], [0, 16], [1, SPAN]])
                    ob = bass.AP(offt.tensor, offt.offset + k0,
                                 [offt.ap[0], [1, 16], [0, SPAN]])
                    nc.vector.tensor_tensor(out=g16[:], in0=ia, in1=ob,
                                            op=mybir.AluOpType.is_equal)
                if qr == 0:
                    ps = pp.tile([128, CIMG], dt.float32, tag="ps", name="ps")
                for q in range(QV):
                    k = k0 + q
                    nc.tensor.matmul(ps[qr * 32:(qr + 1) * 32, :],
                                     g16[:, k % 16, :], pbt[:, k % BQ, :],
                                     start=(q == 0), stop=(q == QV - 1),
                                     tile_position=(0, qr * 32))
                if qr == 3:
                    b = w // 4
                    if b % OB == 0:
                        ot = iop.tile([128, OB, CIMG], dt.bfloat16,
                                      tag="ot", name="ot")
                    nc.scalar.activation(ot[:, b % OB, :], ps[:],
                                         mybir.ActivationFunctionType.Copy)
                    if b % OB == OB - 1 or b == NW4 - 1:
                        b0 = (b // OB) * OB
                        nc.sync.dma_start(out=povirt[:, b0:b + 1, :],
                                          in_=ot[:, 0:b + 1 - b0, :])
    nc.compile()
    return nc


# ---------------------------------------------------------------- launch C
C_OUT_ROWS = 23              # ds2-out rows per core (8*23 = 184 >= 180)


def build_launch_c():
    nc = bacc.Bacc("TRN2", target_bir_lowering=False, debug=False,
                   num_devices=NCORES)
    NR1 = C_OUT_ROWS + 2                         # ds1-out rows incl halo (25)
    NRP = 2 * NR1 + 1                            # pooled rows needed (51)
    slab = nc.dram_tensor("slab", [CIMG, NRP, 362], dt.bfloat16,
                          kind="ExternalInput").ap()
    m1 = nc.dram_tensor("m1", [128, NR1], dt.bfloat16, kind="ExternalInput").ap()
    wd1 = nc.dram_tensor("wd1", [9, CIMG, CIMG], dt.bfloat16,
                         kind="ExternalInput").ap()
    wd2 = nc.dram_tensor("wd2", [9, CIMG, CIMG], dt.bfloat16,
                         kind="ExternalInput").ap()
    sb1 = nc.dram_tensor("sb1", [CIMG, 2], dt.float32, kind="ExternalInput").ap()
    sb2 = nc.dram_tensor("sb2", [CIMG, 2], dt.float32, kind="ExternalInput").ap()
    yout = nc.dram_tensor("yout", [CIMG, C_OUT_ROWS, 180], dt.float32,
                          kind="ExternalOutput").ap()
    RELU = mybir.ActivationFunctionType.Relu
    with tile.TileContext(nc) as tc:
        with tc.tile_pool(name="const", bufs=1) as cpool, \
             tc.tile_pool(name="work", bufs=2) as wp, \
             tc.tile_pool(name="big", bufs=1) as bp, \
             tc.tile_pool(name="ps", bufs=4, space="PSUM") as pp:
            slabt = bp.tile([CIMG, NRP, 362], dt.bfloat16, name="slabt")
            for rr in range(0, NRP, 7):
                nrr = min(7, NRP - rr)
                nc.sync.dma_start(out=slabt[:, rr:rr + nrr, :],
                                  in_=slab[:, rr:rr + nrr, :])
            w1 = cpool.tile([CIMG, 9, CIMG], dt.bfloat16, name="w1")
            nc.sync.dma_start(out=w1[:], in_=wd1.rearrange("t p o -> p t o"))
            w2 = cpool.tile([CIMG, 9, CIMG], dt.bfloat16, name="w2")
            nc.sync.dma_start(out=w2[:], in_=wd2.rearrange("t p o -> p t o"))
            sb1t = cpool.tile([CIMG, 2], dt.float32, name="sb1t")
            nc.sync.dma_start(out=sb1t[:], in_=sb1)
            sb2t = cpool.tile([CIMG, 2], dt.float32, name="sb2t")
            nc.sync.dma_start(out=sb2t[:], in_=sb2)
            m1t = wp.tile([128, NR1], dt.bfloat16, name="m1t")
            nc.sync.dma_start(out=m1t[:], in_=m1)
            h1 = bp.tile([CIMG, NR1, 182], dt.bfloat16, name="h1")
            nc.vector.memset(h1[:, :, 0:1], 0.0)
            nc.vector.memset(h1[:, :, 181:182], 0.0)
            # ds1: stride-2 3x3; out row t reads slab rows 2t..2t+2 (slab row 0
            # = pooled row 2o0-3, so out row t (global o0-1+t) reads
            # 2(o0-1+t)-1..+1 - (2o0-3) = 2t..2t+2); col c reads 2c..2c+2
            RP = 2
            for t0 in range(0, NR1, RP):
                nr = min(RP, NR1 - t0)
                ps = pp.tile([CIMG, nr, 180], dt.float32, tag="ps1", name="ps")
                gi = 0
                for ky in range(3):
                    for kx in range(3):
                        rhs = bass.AP(slabt.tensor,
                                      slabt.offset + (2 * t0 + ky) * 362 + kx,
                                      [slabt.ap[0], [2 * 362, nr], [2, 180]])
                        nc.tensor.matmul(ps[:], w1[:, ky * 3 + kx, :], rhs,
                                         start=(gi == 0), stop=(gi == 8))
                        gi += 1
                nc.scalar.activation(h1[:, t0:t0 + nr, 1:181], ps[:], RELU,
                                     bias=sb1t[:, 1:2], scale=sb1t[:, 0:1])
            # zero rows outside the global [0,180) output range (edges only)
            mlo = bass.AP(m1t.tensor, m1t.offset,
                          [[m1t.ap[0][0], CIMG], [1, 1], [0, 182]])
            nc.vector.tensor_tensor(out=h1[:, 0:1, :], in0=h1[:, 0:1, :],
                                    in1=mlo, op=mybir.AluOpType.mult)
            mhi = bass.AP(m1t.tensor, m1t.offset + NR1 - 5,
                          [[m1t.ap[0][0], CIMG], [1, 5], [0, 182]])
            nc.vector.tensor_tensor(out=h1[:, NR1 - 5:NR1, :],
                                    in0=h1[:, NR1 - 5:NR1, :],
                                    in1=mhi, op=mybir.AluOpType.mult)
            # ds2: 3x3 pad 1: out row o reads h1 rows o..o+2, col c: c..c+2
            yo = bp.tile([CIMG, C_OUT_ROWS, 180], dt.float32, name="yo")
            for o0 in range(0, C_OUT_ROWS, RP):
                nr = min(RP, C_OUT_ROWS - o0)
                ps = pp.tile([CIMG, nr, 180], dt.float32, tag="ps2", name="ps")
                gi = 0
                for ky in range(3):
                    for kx in range(3):
                        rhs = bass.AP(h1.tensor,
                                      h1.offset + (o0 + ky) * 182 + kx,
                                      [h1.ap[0], [182, nr], [1, 180]])
                        nc.tensor.matmul(ps[:], w2[:, ky * 3 + kx, :], rhs,
                                         start=(gi == 0), stop=(gi == 8))
                        gi += 1
                nc.scalar.activation(yo[:, o0:o0 + nr, :], ps[:], RELU,
                                     bias=sb2t[:, 1:2], scale=sb2t[:, 0:1])
                nc.sync.dma_start(out=yout[:, o0:o0 + nr, :],
                                  in_=yo[:, o0:o0 + nr, :])
    nc.compile()
    return nc


_CACHE = {}


def run_launch_a(inputs):
    if "A" not in _CACHE:
        _CACHE["A"] = build_launch_a()
    nc = _CACHE["A"]
    maps = _prep_a_inputs(inputs)
    res = run_bass_kernel_spmd(nc, maps, list(range(NCORES)))
    depth = np.zeros((NPIX, DD), np.float32)
    feat = np.zeros((NPIX, CIMG), np.float32)
    for c in range(NCORES):
        r = res.results[c]
        for s, (cam, h0) in enumerate([SEG_A[c], SEG_B[c]]):
            S = SEGS[s]
            npix = S["nout"] * FW
            base = (cam * FH + h0) * FW
            off = 0 if s == 0 else 16 * FW
            depth[base:base + npix] = r["out_depth"][off:off + npix]
            feat[base:base + npix] = r["out_feat"][off:off + npix].astype(np.float32)
    return depth, feat


def _build_schedule(flat, kept):
    """Sort kept points by (core, local voxel); emit fixed-quota virtual
    windows of QV*128 points with vox-span < SPAN. Returns per-core schedule
    dicts + W (max window count, rounded to 8)."""
    pts = np.arange(NPTS)
    rem = pts % (DD * FH * FW)
    d_i = rem // (FH * FW)
    col = (pts // (DD * FH * FW)) * (FH * FW) + rem % (FH * FW)
    vox = flat
    vx = (vox // NX).astype(np.int32)

    keep_idx = np.where(kept)[0]
    cnt = np.bincount(vx[keep_idx], minlength=NX)
    order = np.argsort(-cnt, kind="stable")
    core_of_row = np.zeros(NX, np.int32)
    load = np.zeros(NCORES, np.int64)
    for r in order:
        c = int(np.argmin(load))
        core_of_row[r] = c
        load[c] += cnt[r]

    row_rank = np.zeros(NX, np.int64)
    rows_of = []
    for c in range(NCORES):
        rows = np.where(core_of_row == c)[0]
        rows_of.append(rows)
        row_rank[rows] = np.arange(len(rows))

    schedules = []
    for c in range(NCORES):
        sel = keep_idx[core_of_row[vx[keep_idx]] == c]
        vloc = row_rank[vx[sel]] * NX + (vox[sel] % NX)
        o = np.argsort(vloc, kind="stable")
        sel, vloc = sel[o], vloc[o]
        win = []                      # (start, end, base)
        i, n = 0, len(sel)
        while i < n:
            base = vloc[i]
            j = min(i + QV * 128, n)
            hi = np.searchsorted(vloc, base + SPAN, "left")
            j = min(j, hi)
            win.append((i, j, base))
            i = j
        schedules.append(dict(sel=sel, vloc=vloc, win=win, col=col[sel],
                              d_i=d_i[sel], rows=rows_of[c]))
    W = max(len(s["win"]) for s in schedules)
    W = (W + 7) // 8 * 8
    return schedules, W


def _prep_b_inputs(schedules, W, depth_rows, featflat):
    maps = []
    NCH = W * QV
    iota = np.broadcast_to(np.arange(SPAN, dtype=np.float32)[None, :],
                           (128, SPAN)).astype(bf16)
    for sch in schedules:
        pb = np.zeros((128, NCH, CIMG), bf16)
        offv = np.zeros((128, NCH), bf16)
        col, d_i, vloc = sch["col"], sch["d_i"], sch["vloc"]
        dvals = depth_rows[col, d_i]                  # f32 depth weights
        wfeat = (dvals[:, None] * featflat[col]).astype(bf16)
        for w, (i, j, base) in enumerate(sch["win"]):
            L = j - i
            nch = (L + 127) // 128
            for q in range(nch):
                lo, hi = q * 128, min((q + 1) * 128, L)
                k = w * QV + q
                pb[0:hi - lo, k] = wfeat[i + lo:i + hi]
                offv[0:hi - lo, k] = (vloc[i + lo:i + hi] - base).astype(
                    np.float32)
        maps.append(dict(pb=pb, offv=offv, iotain=iota))
    return maps


def _prep_c_inputs(inputs, pooled_t):
    """pooled_t: [CIMG, 360, 360] f32 -> per-core slabs + masks + weights."""
    NR1 = C_OUT_ROWS + 2
    NRP = 2 * NR1 + 1
    w1 = np.asarray(inputs["ds1_w"], np.float32)
    w2 = np.asarray(inputs["ds2_w"], np.float32)
    wd1 = np.stack([w1[:, :, ky, kx].T for ky in range(3) for kx in range(3)])
    wd2 = np.stack([w2[:, :, ky, kx].T for ky in range(3) for kx in range(3)])
    sb1 = np.stack([np.asarray(inputs["ds1_s"], np.float32),
                    np.asarray(inputs["ds1_t"], np.float32)], 1)
    sb2 = np.stack([np.asarray(inputs["ds2_s"], np.float32),
                    np.asarray(inputs["ds2_t"], np.float32)], 1)
    shared = dict(wd1=wd1.astype(bf16), wd2=wd2.astype(bf16), sb1=sb1, sb2=sb2)
    maps = []
    pt_bf = pooled_t.astype(bf16)
    for c in range(NCORES):
        o0g = C_OUT_ROWS * c
        p0 = 2 * o0g - 3
        slab = np.zeros((CIMG, NRP, 362), bf16)
        lo, hi = max(0, p0), min(NX, p0 + NRP)
        if hi > lo:
            slab[:, lo - p0:hi - p0, 1:361] = pt_bf[:, lo:hi, :]
        t1g = np.arange(NR1) + (o0g - 1)
        m1 = np.broadcast_to(((t1g >= 0) & (t1g < 180))[None, :],
                             (128, NR1)).astype(bf16)
        maps.append(dict(shared, slab=slab, m1=np.ascontiguousarray(m1)))
    return maps


def kernel(**inputs):
    inputs = {k: np.asarray(v) for k, v in inputs.items()}
    flat, kept = _host_geometry(inputs["cam2lidar_rots"],
                                inputs["cam2lidar_trans"], inputs["intrins"],
                                inputs["post_rots"], inputs["post_trans"])
    depth_rows, feat_rows = run_launch_a(inputs)

    schedules, W = _build_schedule(flat, kept)
    key = ("B", W)
    if key not in _CACHE:
        _CACHE[key] = build_launch_b(W)
    bmaps = _prep_b_inputs(schedules, W, depth_rows, feat_rows)
    res_b = run_bass_kernel_spmd(_CACHE[key], bmaps, list(range(NCORES)))

    pooled = np.zeros((NX * NX, CIMG), np.float32)
    for c, sch in enumerate(schedules):
        virt = res_b.results[c]["povirt"].astype(np.float32)  # [128, NW4, C]
        rows_arr = sch["rows"]
        nloc = len(rows_arr) * NX
        for w, (i, j, base) in enumerate(sch["win"]):
            span = min(SPAN, nloc - base)
            lidx = base + np.arange(span)
            ridx = rows_arr[lidx // NX] * NX + (lidx % NX)
            qr = w % 4
            pooled[ridx] += virt[qr * 32:qr * 32 + span, w // 4]
    pooled_t = np.ascontiguousarray(
        pooled.reshape(NX, NX, CIMG).transpose(2, 0, 1))

    if "C" not in _CACHE:
        _CACHE["C"] = build_launch_c()
    cmaps = _prep_c_inputs(inputs, pooled_t)
    res_c = run_bass_kernel_spmd(_CACHE["C"], cmaps, list(range(NCORES)))
    out = np.zeros((1, CIMG, 180, 180), np.float32)
    for c in range(NCORES):
        o0g = C_OUT_ROWS * c
        nr = min(C_OUT_ROWS, 180 - o0g)
        if nr > 0:
            out[0, :, o0g:o0g + nr, :] = res_c.results[c]["yout"][:, 0:nr, :]
    return out



# revision 19
# speedup vs baseline: 1.4473x; 1.0831x over previous
"""DepthLSSTransform Trainium kernel: 3 SPMD launches over 8 NeuronCores.

Launch A: per-camera conv pipeline (dtransform + depthnet + softmax) on
          24-row bands (one 16-row + one 8-row segment per core).
Launch B: bev_pool segment-sum via one-hot matmuls over a host-built
          virtual-window schedule (sorted-by-voxel points).
Launch C: BEV downsample convs, spatially sharded.
Host: geometry/voxel indices, scheduling, gathers, folds (orchestration).
"""
import numpy as np
import ml_dtypes

import concourse.bass as bass
import concourse.tile as tile
from concourse import bacc, mybir
from concourse.bass_utils import run_bass_kernel_spmd

dt = mybir.dt
bf16 = ml_dtypes.bfloat16

# ---- problem constants (hardcoded per contract) ----
B, N = 1, 6
CIN, CIMG, DD = 256, 80, 59
FH, FW, IH, IW = 32, 88, 256, 704
XY0, DXY, NX = -54.0, 0.3, 360
Z0, DZ, NZ = -10.0, 20.0, 1
NPTS = N * DD * FH * FW
NPIX = N * FH * FW
NCORES = 8
QV = 4                      # chunks of 128 points per virtual window

# per-core segments: (camera, h0) for seg A (16 rows) and seg B (8 rows)
SEG_A = [(0, 0), (1, 0), (1, 16), (2, 16), (3, 0), (4, 0), (4, 16), (5, 16)]
SEG_B = [(0, 16), (0, 24), (2, 0), (2, 8), (3, 16), (3, 24), (5, 0), (5, 8)]
# band pixel ranges in global row order (row = n*32 + h)
ROWS_OF_CORE = [[(SEG_A[c][0] * FH + SEG_A[c][1] + r) for r in range(16)] +
                [(SEG_B[c][0] * FH + SEG_B[c][1] + r) for r in range(8)]
                for c in range(NCORES)]

# segment geometry: rows16 segment: d rows [8h0-34, 8h0+158) (192), dt2 out
# rows [2h0-8, 2h0+39) (47), dt3 [h0-3, h0+19) (22), dn1 [h0-1, h0+17) (18)
SEGS = [dict(nout=16, nd=192, nq=48, nt2=47, nt3=22, nn1=18),
        dict(nout=8, nd=128, nq=32, nt2=31, nt3=14, nn1=10)]


def _seg_ranges(h0, S):
    return dict(d0=8 * h0 - 34, q0=2 * h0 - 8, t0=h0 - 3, r0=h0 - 1, o0=h0)


# ---------------------------------------------------------------- launch A
# edge-mask layout per segment: dt2 rows [0:8]+[nt2-7:nt2], dt3 rows
# [0:3]+[nt3-3:nt3], dn1 rows [0:1]+[nn1-1:nn1]  -> 23 columns
EMSK = 23


def build_launch_a(debug=False, psum_bufs=4, work_bufs=3, stages=9):
    nc = bacc.Bacc("TRN2", target_bir_lowering=False, debug=False,
                   num_devices=NCORES)
    AP = {}

    def inp(name, shape, dtype=dt.bfloat16):
        AP[name] = nc.dram_tensor(name, shape, dtype, kind="ExternalInput").ap()
        return AP[name]

    # per segment inputs (s = 0: 16-row, 1: 8-row)
    for s, S in enumerate(SEGS):
        inp(f"dph{s}", [128, S["nq"], 177])             # poison-filled OOB
        inp(f"masks{s}", [128, EMSK])                   # edge-row masks
        inp(f"xseg{s}", [CIN, S["nt3"], FW])            # x_img slice (zeroed oob)
    # packed f32 constants: [alpha, beta, s_dt2, t_dt2, s_dt3, t_dt3,
    #  s_dn1(2), t_dn1(2), s_dn2(2), t_dn2(2), b_dn3(139)] -> [128, 153]
    inp("consts", [128, 153], dt.float32)
    inp("bias3", [1, 139])                              # dn3 bias row bf16
    # conv weights (host-prepped layouts)
    inp("w_dt2", [4, 128, 32])                          # groups (dky,dmx)
    inp("w_dt3", [9, 128, 64])
    inp("w_dn1", [9, 3, 128, 256])                      # tap, icchunk(128,128,64pad) -> 256
    inp("w_dn2", [9, 2, 128, 256])
    inp("w_dn3", [2, 128, 139])

    DBG = {}
    dbg_specs = [] if not debug else [("dbg_t1", [128, SEGS[0]["nq"], 177], dt.bfloat16),
                        ("dbg_dt2o", [32, SEGS[0]["nt2"] + 1, 180], dt.bfloat16),
                        ("dbg_dtc", [64, SEGS[0]["nt3"], 92], dt.bfloat16),
                        ("dbg_n1o", [128, SEGS[0]["nn1"], 92], dt.bfloat16),
                        ("dbg_n2o", [128, SEGS[0]["nout"], 88], dt.bfloat16)]
    for nm, sh, dty in dbg_specs:
        DBG[nm] = nc.dram_tensor(nm, sh, dty, kind="ExternalOutput").ap()
    # chunk-major outputs: pixel = (seg base) + pc*128 + p
    NPC = [(SEGS[0]["nout"] * FW + 127) // 128, (SEGS[1]["nout"] * FW + 127) // 128]
    out_depth = nc.dram_tensor("out_depth", [128, NPC[0] + NPC[1], DD],
                               dt.float32, kind="ExternalOutput").ap()
    out_feat = nc.dram_tensor("out_feat", [128, NPC[0] + NPC[1], CIMG],
                              dt.bfloat16, kind="ExternalOutput").ap()

    # HBM scratch
    scr = {}
    for s, S in enumerate(SEGS):
        scr[f"dt2o{s}"] = nc.dram_tensor(f"dt2o{s}", [32, S["nt2"] + 1, 2, 90], dt.bfloat16).ap()

    RELU = mybir.ActivationFunctionType.Relu
    with tile.TileContext(nc) as tc:
        with tc.tile_pool(name="const", bufs=1) as cpool, \
             tc.tile_pool(name="work", bufs=work_bufs) as wpool, \
             tc.tile_pool(name="big", bufs=1) as bpool, \
             tc.tile_pool(name="psum", bufs=psum_bufs, space="PSUM") as ppool:
            # ---- load packed constants in one DMA ----
            cts = cpool.tile([128, 153], dt.float32, name="cts")
            nc.sync.dma_start(out=cts[:], in_=AP["consts"])
            ct = {"dt1_alpha": cts[:, 0:1], "dt1_beta": cts[:, 1:2],
                  "s_dt2": cts[:, 2:3], "t_dt2": cts[:, 3:4],
                  "s_dt3": cts[:, 4:5], "t_dt3": cts[:, 5:6],
                  "s_dn1": cts[:, 6:8], "t_dn1": cts[:, 8:10],
                  "s_dn2": cts[:, 10:12], "t_dn2": cts[:, 12:14],
                  "b_dn3": cts[:, 14:153]}
            # small weights first (dt2/dt3 unblock the pipeline head)
            wt = {}

            def load_w(nm, pat):
                sh = list(AP[nm].shape)
                wt[nm] = cpool.tile([sh[-2], int(np.prod(sh[:-2])), sh[-1]],
                                    dt.bfloat16, tag=nm, name=f'wt_{nm}')
                nc.sync.dma_start(out=wt[nm][:], in_=AP[nm].rearrange(pat))

            load_w("w_dt2", "g p o -> p g o")
            load_w("w_dt3", "g p o -> p g o")

            # stage inputs for both segments before the big dn weights
            dphs, malls = {}, {}
            for s, S in enumerate(SEGS):
                nq = S["nq"]
                dphs[s] = bpool.tile([128, nq, 177], dt.bfloat16, tag=f"dph{s}", name=f"dph{s}")
                for qq in range(0, nq, nq // 4):
                    nqq = min(nq // 4, nq - qq)
                    nc.sync.dma_start(out=dphs[s][:, qq:qq + nqq, :],
                                      in_=AP[f"dph{s}"][:, qq:qq + nqq, :])
                malls[s] = wpool.tile([128, EMSK], dt.bfloat16,
                                      tag=f"msk{s}", name="mall")
                nc.sync.dma_start(out=malls[s][:], in_=AP[f"masks{s}"])

            load_w("w_dn1", "t i p o -> p (t i) o")
            load_w("w_dn2", "t i p o -> p (t i) o")
            load_w("w_dn3", "g p o -> p g o")
            b3t = cpool.tile([1, 139], dt.bfloat16, name="b3t")
            nc.sync.dma_start(out=b3t[:], in_=AP["bias3"])
            ones1 = cpool.tile([1, 128], dt.bfloat16, name="ones1")
            nc.vector.memset(ones1[:], 1.0)

            feat_sb = {}
            depth_sb = {}
            for s, S in enumerate(SEGS):
                nq, nt2, nt3, nn1, nout = S["nq"], S["nt2"], S["nt3"], S["nn1"], S["nout"]
                # ==== dt1: relu(alpha*d + beta) on DVE; OOB host-poisoned ====
                dph, mall = dphs[s], malls[s]
                t1 = bpool.tile([128, nq, 177], dt.bfloat16, tag=f"t1{s}")
                QCH = nq // 4
                for qq in range(0, nq, QCH):
                    nqq = min(QCH, nq - qq)
                    sl = (slice(None), slice(qq, qq + nqq), slice(None))
                    nc.vector.tensor_scalar(out=t1[sl], in0=dph[sl],
                                            scalar1=ct["dt1_alpha"][:, 0:1],
                                            scalar2=ct["dt1_beta"][:, 0:1],
                                            op0=mybir.AluOpType.mult,
                                            op1=mybir.AluOpType.add)
                    nc.vector.tensor_scalar(out=t1[sl], in0=t1[sl], scalar1=0.0,
                                            scalar2=None, op0=mybir.AluOpType.max)
                if s == 0 and debug:
                    nc.sync.dma_start(out=DBG["dbg_t1"], in_=t1[:])

                if stages < 2:
                    continue
                # ======== dt2 ========
                o2 = bpool.tile([32, nt2 + 1, 180], dt.bfloat16, tag=f"o2{s}")
                nc.vector.memset(o2[:, :, 0:1], 0.0)
                nc.vector.memset(o2[:, :, 89:91], 0.0)
                nc.vector.memset(o2[:, :, 179:180], 0.0)
                nc.vector.memset(o2[:, nt2:nt2 + 1, :], 0.0)
                RPP2 = 2
                for q0 in range(0, nt2, RPP2):
                    nr = min(RPP2, nt2 - q0)
                    ps = ppool.tile([32, nr, 176], dt.float32, tag=f"ps{s}", name="ps2")
                    gi = 0
                    for dky in range(2):
                        for dmx in range(2):
                            g = dky * 2 + dmx
                            rhs = bass.AP(
                                t1.tensor, t1.offset + (q0 + dky) * 177 + dmx,
                                [t1.ap[0], [177, nr], [1, 176]])
                            nc.tensor.matmul(ps[:], wt["w_dt2"][:, g, :], rhs,
                                             start=(gi == 0), stop=(gi == 3))
                            gi += 1
                    # write col c at (c%2)*90 + c//2 + 1  (phase-split layout)
                    o2dst = bass.AP(o2.tensor, o2.offset + q0 * 180 + 1,
                                    [[o2.ap[0][0], 32], [180, nr],
                                     [1, 88], [90, 2]])
                    nc.scalar.activation(o2dst, ps[:], RELU,
                                         bias=ct["t_dt2"][0:32, 0:1],
                                         scale=ct["s_dt2"][0:32, 0:1])
                # zero image-OOB edge rows (masks: interior cores all-ones)
                mlo = bass.AP(mall.tensor, mall.offset,
                              [[mall.ap[0][0], 32], [1, 8], [0, 180]])
                nc.vector.tensor_tensor(out=o2[:, 0:8, :], in0=o2[:, 0:8, :],
                                        in1=mlo, op=mybir.AluOpType.mult)
                mhi = bass.AP(mall.tensor, mall.offset + 8,
                              [[mall.ap[0][0], 32], [1, 7], [0, 180]])
                nc.vector.tensor_tensor(out=o2[:, nt2 - 7:nt2, :],
                                        in0=o2[:, nt2 - 7:nt2, :],
                                        in1=mhi, op=mybir.AluOpType.mult)
                nc.sync.dma_start(out=scr[f"dt2o{s}"],
                                  in_=o2.rearrange("p q (b x) -> p q b x", b=2))
                if s == 0 and debug:
                    nc.sync.dma_start(out=DBG["dbg_dt2o"], in_=o2[:])

                if stages < 3:
                    continue
                # ======== dt3 ========
                nry3 = nt3 + 2
                ph3 = bpool.tile([128, nry3, 90], dt.bfloat16, tag=f"ph3{s}")
                sd2 = scr[f"dt2o{s}"]
                for a2 in range(2):
                    for b2 in range(2):
                        pap3 = bass.AP(sd2.tensor,
                                       sd2.offset + a2 * 180 + b2 * 90,
                                       [[(nt2 + 1) * 180, 32],
                                        [2 * 180, nry3], [1, 90]])
                        nc.sync.dma_start(
                            out=ph3[(a2 * 2 + b2) * 32:(a2 * 2 + b2 + 1) * 32],
                            in_=pap3)
                # concat input tile: [64 dt3 | pad] plus x_img tiles
                dtc = bpool.tile([64, nt3, 92], dt.bfloat16, tag=f"dtc{s}")
                nc.vector.memset(dtc[:, :, 0:2], 0.0)
                nc.vector.memset(dtc[:, :, 90:92], 0.0)
                RPP3 = 4
                for t0 in range(0, nt3, RPP3):
                    nr = min(RPP3, nt3 - t0)
                    ps = ppool.tile([64, nr, 88], dt.float32, tag=f"ps{s}")
                    gi = 0
                    for dky in range(3):
                        for dmx in range(3):
                            g = dky * 3 + dmx
                            rhs = bass.AP(ph3.tensor,
                                          ph3.offset + (t0 + dky) * 90 + dmx,
                                          [ph3.ap[0], [90, nr], [1, 88]])
                            nc.tensor.matmul(ps[:], wt["w_dt3"][:, g, :], rhs,
                                             start=(gi == 0), stop=(gi == 8))
                            gi += 1
                    nc.scalar.activation(dtc[:, t0:t0 + nr, 2:90], ps[:], RELU,
                                         bias=ct["t_dt3"][0:64, 0:1],
                                         scale=ct["s_dt3"][0:64, 0:1])
                m3lo = bass.AP(mall.tensor, mall.offset + 15,
                               [[mall.ap[0][0], 64], [1, 3], [0, 92]])
                nc.vector.tensor_tensor(out=dtc[:, 0:3, :], in0=dtc[:, 0:3, :],
                                        in1=m3lo, op=mybir.AluOpType.mult)
                m3hi = bass.AP(mall.tensor, mall.offset + 18,
                               [[mall.ap[0][0], 64], [1, 3], [0, 92]])
                nc.vector.tensor_tensor(out=dtc[:, nt3 - 3:nt3, :],
                                        in0=dtc[:, nt3 - 3:nt3, :],
                                        in1=m3hi, op=mybir.AluOpType.mult)

                if s == 0 and debug:
                    nc.sync.dma_start(out=DBG["dbg_dtc"], in_=dtc[:])
                if stages < 4:
                    continue
                # ======== dn1 ========
                xs = []
                for g in range(2):
                    xt = bpool.tile([128, nt3, 92], dt.bfloat16, tag=f"x{g}_{s}",
                                     name=f"xseg_t{g}")
                    nc.vector.memset(xt[:, :, 0:2], 0.0)
                    nc.vector.memset(xt[:, :, 90:92], 0.0)
                    nc.sync.dma_start(
                        out=xt[:, :, 2:90],
                        in_=AP[f"xseg{s}"][g * 128:(g + 1) * 128])
                    xs.append(xt)
                n1o = []
                for g in range(2):
                    t = bpool.tile([128, nn1, 92], dt.bfloat16, tag=f"n1o{g}_{s}")
                    nc.vector.memset(t[:, :, 0:2], 0.0)
                    nc.vector.memset(t[:, :, 90:92], 0.0)
                    n1o.append(t)
                RPP = 5
                for ocg in range(2):
                    for r0 in range(0, nn1, RPP):
                        nr = min(RPP, nn1 - r0)
                        ps = ppool.tile([128, nr, 88], dt.float32, tag=f"ps{s}")
                        gi = 0
                        for ky in range(3):
                            for kx in range(3):
                                tap = ky * 3 + kx
                                for icc, srcT in enumerate((xs[0], xs[1], dtc)):
                                    kk = 128 if icc < 2 else 64
                                    rhs = bass.AP(
                                        srcT.tensor,
                                        srcT.offset + (r0 + ky + 1) * 92 + kx + 1,
                                        [srcT.ap[0], [92, nr], [1, 88]])
                                    lhs = wt["w_dn1"][0:kk, tap * 3 + icc,
                                                      ocg * 128:(ocg + 1) * 128]
                                    nc.tensor.matmul(ps[:], lhs, rhs,
                                                     start=(gi == 0),
                                                     stop=(gi == 26))
                                    gi += 1
                        nc.scalar.activation(n1o[ocg][:, r0:r0 + nr, 2:90],
                                             ps[:], RELU,
                                             bias=ct["t_dn1"][:, ocg:ocg + 1],
                                             scale=ct["s_dn1"][:, ocg:ocg + 1])
                for g in range(2):
                    mnlo = bass.AP(mall.tensor, mall.offset + 21,
                                   [mall.ap[0], [1, 1], [0, 92]])
                    nc.vector.tensor_tensor(out=n1o[g][:, 0:1, :],
                                            in0=n1o[g][:, 0:1, :],
                                            in1=mnlo, op=mybir.AluOpType.mult)
                    mnhi = bass.AP(mall.tensor, mall.offset + 22,
                                   [mall.ap[0], [1, 1], [0, 92]])
                    nc.vector.tensor_tensor(out=n1o[g][:, nn1 - 1:nn1, :],
                                            in0=n1o[g][:, nn1 - 1:nn1, :],
                                            in1=mnhi, op=mybir.AluOpType.mult)

                if s == 0 and debug:
                    nc.sync.dma_start(out=DBG["dbg_n1o"], in_=n1o[0][:])
                if stages < 5:
                    continue
                # ======== dn2 ========
                n2o = []
                for g in range(2):
                    n2o.append(bpool.tile([128, nout, 88], dt.bfloat16,
                                          tag=f"n2o{g}_{s}", name=f"n2o{g}"))
                for ocg in range(2):
                    for r0 in range(0, nout, RPP):
                        nr = min(RPP, nout - r0)
                        ps = ppool.tile([128, nr, 88], dt.float32, tag=f"ps{s}")
                        gi = 0
                        for ky in range(3):
                            for kx in range(3):
                                tap = ky * 3 + kx
                                for icc in range(2):
                                    rhs = bass.AP(
                                        n1o[icc].tensor,
                                        n1o[icc].offset + (r0 + ky) * 92 + kx + 1,
                                        [n1o[icc].ap[0], [92, nr], [1, 88]])
                                    lhs = wt["w_dn2"][:, tap * 2 + icc,
                                                      ocg * 128:(ocg + 1) * 128]
                                    nc.tensor.matmul(ps[:], lhs, rhs,
                                                     start=(gi == 0),
                                                     stop=(gi == 17))
                                    gi += 1
                        nc.scalar.activation(n2o[ocg][:, r0:r0 + nr, :], ps[:],
                                             RELU,
                                             bias=ct["t_dn2"][:, ocg:ocg + 1],
                                             scale=ct["s_dn2"][:, ocg:ocg + 1])

                if s == 0 and debug:
                    nc.sync.dma_start(out=DBG["dbg_n2o"], in_=n2o[0][:])
                if stages < 6:
                    continue
                # ======== dn3 + softmax + feat ========
                npix = nout * FW
                feat_sb[s] = bpool.tile([128, (npix + 127) // 128, CIMG],
                                        dt.bfloat16, tag=f"feat{s}", name=f"feat_sb{s}")
                depth_sb[s] = bpool.tile([128, (npix + 127) // 128, DD],
                                         dt.float32, tag=f"depth{s}", name=f"depth_sb{s}")
                n2f = [t.rearrange("p a b -> p (a b)") for t in n2o]
                for pc in range((npix + 127) // 128):
                    m = min(128, npix - pc * 128)
                    ps = ppool.tile([m, 139], dt.float32, tag=f"ps{s}")
                    for icc in range(2):
                        nc.tensor.matmul(ps[:], n2f[icc][:, pc * 128:pc * 128 + m],
                                         wt["w_dn3"][:, icc, :],
                                         start=(icc == 0), stop=False)
                    # bias row via rank-1 matmul: ones[1,m].T @ b3t[1,139]
                    nc.tensor.matmul(ps[:], ones1[0:1, 0:m], b3t[:],
                                     start=False, stop=True)
                    # softmax over first 59 channels, straight from PSUM
                    mx = wpool.tile([m, 1], dt.float32, tag=f"mx{s}")
                    nc.vector.reduce_max(mx[:], ps[:, 0:DD],
                                         axis=mybir.AxisListType.X, negate=True)
                    ex = wpool.tile([m, DD], dt.float32, tag=f"ex{s}")
                    nc.scalar.activation(ex[:], ps[:, 0:DD],
                                         mybir.ActivationFunctionType.Exp,
                                         bias=mx[:, 0:1], scale=1.0)
                    sm = wpool.tile([m, 1], dt.float32, tag=f"sm{s}")
                    nc.vector.reduce_sum(sm[:], ex[:], axis=mybir.AxisListType.X)
                    rc = wpool.tile([m, 1], dt.float32, tag=f"rc{s}")
                    nc.vector.reciprocal(rc[:], sm[:])
                    nc.vector.tensor_scalar(out=depth_sb[s][0:m, pc, :], in0=ex[:],
                                            scalar1=rc[:, 0:1], scalar2=None,
                                            op0=mybir.AluOpType.mult)
                    nc.vector.tensor_copy(feat_sb[s][0:m, pc, :],
                                          ps[:, DD:DD + CIMG])

            # DMA outputs, chunk-major (contiguous per-partition lines)
            for s, S in (enumerate(SEGS) if stages >= 6 else []):
                b0 = 0 if s == 0 else NPC[0]
                nc.sync.dma_start(out=out_depth[:, b0:b0 + NPC[s], :],
                                  in_=depth_sb[s][:])
                nc.sync.dma_start(out=out_feat[:, b0:b0 + NPC[s], :],
                                  in_=feat_sb[s][:])
    nc.compile()
    return nc


# ------------------------------------------------------------ host helpers
def _host_geometry(rots, trans, intr, post_rots, post_trans):
    import jax
    import jax.numpy as jnp
    with jax.default_device(jax.devices("cpu")[0]):
        f32 = jnp.float32
        ds = jnp.arange(1.0, 60.0, 1.0, dtype=f32)
        xs = jnp.linspace(0.0, IW - 1.0, FW, dtype=f32)
        ys = jnp.linspace(0.0, IH - 1.0, FH, dtype=f32)
        dm = jnp.broadcast_to(ds[:, None, None], (DD, FH, FW))
        xm = jnp.broadcast_to(xs[None, None, :], (DD, FH, FW))
        ym = jnp.broadcast_to(ys[None, :, None], (DD, FH, FW))
        fr = jnp.stack([xm, ym, dm], -1)
        pts = fr[None, None] - jnp.asarray(post_trans)[:, :, None, None, None, :]
        pts = jnp.einsum("bnij,bndhwj->bndhwi",
                         jnp.linalg.inv(jnp.asarray(post_rots)), pts)
        pts = jnp.concatenate([pts[..., :2] * pts[..., 2:3], pts[..., 2:3]], -1)
        comb = jnp.einsum("bnij,bnjk->bnik", jnp.asarray(rots),
                          jnp.linalg.inv(jnp.asarray(intr)))
        pts = jnp.einsum("bnij,bndhwj->bndhwi", comb, pts) \
            + jnp.asarray(trans)[:, :, None, None, None, :]
        lo = jnp.array([XY0, XY0, Z0], dtype=f32)
        dxv = jnp.array([DXY, DXY, DZ], dtype=f32)
        g = ((pts - lo) / dxv).astype(jnp.int32).reshape(-1, 3)
        kept = ((g[:, 0] >= 0) & (g[:, 0] < NX) & (g[:, 1] >= 0) & (g[:, 1] < NX)
                & (g[:, 2] >= 0) & (g[:, 2] < NZ))
        flat = (g[:, 2] * NX + g[:, 0]) * NX + g[:, 1]
        return np.asarray(flat, np.int64), np.asarray(kept)


def _prep_a_inputs(inputs):
    """Build per-core input maps for launch A."""
    d = np.asarray(inputs["d"], np.float32).reshape(N, IH, IW)
    x_img = np.asarray(inputs["x_img"], np.float32)

    # dt1 folded affine: relu(alpha*d + beta), alpha = s*w, beta = s*b + t
    a1 = (inputs["dt1_s"] * inputs["dt1_w"][:, 0, 0, 0]).astype(np.float32)
    b1 = (inputs["dt1_s"] * inputs["dt1_b"] + inputs["dt1_t"]).astype(np.float32)
    cab = np.arange(128)
    dt1_alpha = a1[cab // 16][:, None]
    dt1_beta = b1[cab // 16][:, None]

    def wprep_dt2():
        w = np.asarray(inputs["dt2_w"], np.float32)      # [32,8,5,5]
        out = np.zeros((4, 128, 32), np.float32)
        for ky in range(5):
            for kx in range(5):
                a, dky = ky % 4, ky // 4
                bph, dmx = (kx + 2) % 4, (kx + 2) // 4
                g = dky * 2 + dmx
                rows = (np.arange(8)) * 16 + a * 4 + bph
                out[g, rows, :] = w[:, :, ky, kx].T
        return out.astype(bf16)

    def wprep_dt3():
        w = np.asarray(inputs["dt3_w"], np.float32)      # [64,32,5,5]
        out = np.zeros((9, 128, 64), np.float32)
        for ky in range(5):
            for kx in range(5):
                a, dky = ky % 2, ky // 2
                bph, dmx = kx % 2, (kx + 2) // 2 - 1
                g = dky * 3 + dmx
                rows = (a * 2 + bph) * 32 + np.arange(32)
                out[g, rows, :] = w[:, :, ky, kx].T
        return out.astype(bf16)

    def wprep_3x3(w, icc_sizes):
        O, I = w.shape[0], w.shape[1]
        nic = len(icc_sizes)
        out = np.zeros((9, nic, 128, O), np.float32)
        for ky in range(3):
            for kx in range(3):
                tap = ky * 3 + kx
                ic0 = 0
                for icc, sz in enumerate(icc_sizes):
                    out[tap, icc, 0:sz, :] = w[:, ic0:ic0 + sz, ky, kx].T
                    ic0 += sz
        return out.astype(bf16)

    # NOTE: dn1 input concat order is [dt3(64) | x_img(256)] in the reference;
    # our matmul chunks are (x0:128, x1:128, dt3:64) -> weight cols must match:
    w_dn1_full = np.asarray(inputs["dn1_w"], np.float32)
    w_dn1 = np.zeros((9, 3, 128, 256), np.float32)
    for ky in range(3):
        for kx in range(3):
            tap = ky * 3 + kx
            w_dn1[tap, 0, :, :] = w_dn1_full[:, 64:192, ky, kx].T
            w_dn1[tap, 1, :, :] = w_dn1_full[:, 192:320, ky, kx].T
            w_dn1[tap, 2, 0:64, :] = w_dn1_full[:, 0:64, ky, kx].T
    w_dn1 = w_dn1.astype(bf16)
    w_dn2 = wprep_3x3(np.asarray(inputs["dn2_w"], np.float32), [128, 128])
    w_dn3 = np.asarray(inputs["dn3_w"], np.float32)[:, :, 0, 0]  # [139, 256]
    w_dn3p = np.zeros((2, 128, 139), np.float32)
    w_dn3p[0] = w_dn3[:, 0:128].T
    w_dn3p[1] = w_dn3[:, 128:256].T

    def fold_bias(b, s, t):
        # conv bias b then bn scale/shift: relu(s*(x+b) + t) = relu(s*x + (s*b+t))
        return np.asarray(s, np.float32), np.asarray(s * b + t, np.float32)

    s2, t2 = fold_bias(inputs["dt2_b"], inputs["dt2_s"], inputs["dt2_t"])
    s3, t3 = fold_bias(inputs["dt3_b"], inputs["dt3_s"], inputs["dt3_t"])
    sn1, tn1 = fold_bias(inputs["dn1_b"], inputs["dn1_s"], inputs["dn1_t"])
    sn2, tn2 = fold_bias(inputs["dn2_b"], inputs["dn2_s"], inputs["dn2_t"])
    b_dn3 = np.broadcast_to(np.asarray(inputs["dn3_b"], np.float32)[None, :],
                            (128, 139)).copy()

    consts = np.zeros((128, 153), np.float32)
    consts[:, 0] = dt1_alpha[:, 0]
    consts[:, 1] = dt1_beta[:, 0]
    consts[:, 2] = np.tile(s2, 4)
    consts[:, 3] = np.tile(t2, 4)
    consts[:, 4] = np.tile(s3, 2)
    consts[:, 5] = np.tile(t3, 2)
    consts[:, 6:8] = sn1.reshape(2, 128).T
    consts[:, 8:10] = tn1.reshape(2, 128).T
    consts[:, 10:12] = sn2.reshape(2, 128).T
    consts[:, 12:14] = tn2.reshape(2, 128).T
    consts[:, 14:153] = b_dn3
    shared = dict(
        consts=consts,
        w_dt2=wprep_dt2(), w_dt3=wprep_dt3(), w_dn1=w_dn1, w_dn2=w_dn2,
        w_dn3=w_dn3p.astype(bf16),
        bias3=np.asarray(inputs["dn3_b"], np.float32)[None, :].astype(bf16),
    )
    # poison for OOB dt1 inputs: relu(alpha*poison + beta) == 0
    pois = np.where(a1 >= 0, -1e30, 1e30).astype(np.float32)

    maps = []
    for c in range(NCORES):
        m = dict(shared)
        for s, (cam, h0) in enumerate([SEG_A[c], SEG_B[c]]):
            S = SEGS[s]
            d0 = 8 * h0 - 34
            dseg = np.full((S["nd"], 712), np.inf, np.float32)
            lo, hi = max(0, d0), min(IH, d0 + S["nd"])
            if hi > lo:
                dseg[lo - d0:hi - d0, 4:708] = d[cam, lo:hi]
            nq = S["nq"]
            ph = dseg.reshape(nq, 4, 178, 4)[:, :, :177, :]     # ry a rx b
            ph = ph.transpose(1, 3, 0, 2)                        # a b ry rx
            dphf = np.broadcast_to(ph[None], (8, 4, 4, nq, 177)).copy()
            dphf = np.where(np.isfinite(dphf), dphf,
                            pois[:, None, None, None, None])
            m[f"dph{s}"] = dphf.reshape(128, nq, 177).astype(bf16)
            # edge-row masks (image-boundary zeroing after each conv)
            q0, t0, r0 = 2 * h0 - 8, h0 - 3, h0 - 1
            q_lo = q0 + np.arange(8)
            q_hi = q0 + (S["nt2"] - 7) + np.arange(7)
            t_lo = t0 + np.arange(3)
            t_hi = t0 + (S["nt3"] - 3) + np.arange(3)
            r_lo = r0 + np.arange(1)
            r_hi = r0 + (S["nn1"] - 1) + np.arange(1)
            em = np.concatenate([
                (q_lo >= 0) & (q_lo < 64), (q_hi >= 0) & (q_hi < 64),
                (t_lo >= 0) & (t_lo < FH), (t_hi >= 0) & (t_hi < FH),
                (r_lo >= 0) & (r_lo < FH), (r_hi >= 0) & (r_hi < FH)])
            m[f"masks{s}"] = np.broadcast_to(
                em[None, :], (128, EMSK)).astype(bf16)
            xseg = np.zeros((CIN, S["nt3"], FW), np.float32)
            lo2, hi2 = max(0, t0), min(FH, t0 + S["nt3"])
            if hi2 > lo2:
                xseg[:, lo2 - t0:hi2 - t0, :] = x_img[cam, :, lo2:hi2, :]
            m[f"xseg{s}"] = xseg.astype(bf16)
        maps.append(m)
    return maps


# ---------------------------------------------------------------- launch B
SPAN = 32                   # voxel span per window (one-hot width)


def build_launch_b(W):
    """W windows x QV chunks of 128 points; scatter-sum into span-32 windows.

    4 windows share one PSUM bank (partition quarters via tile_position);
    one-hot rows are built 16 chunks per DVE op; depth weights are folded
    into pb on the host."""
    nc = bacc.Bacc("TRN2", target_bir_lowering=False, debug=False,
                   num_devices=NCORES)
    NCH = W * QV                # chunk count (multiple of 32)
    NW4 = W // 4                # psum bank-groups
    pb = nc.dram_tensor("pb", [128, NCH, CIMG], dt.bfloat16,
                        kind="ExternalInput").ap()
    offv = nc.dram_tensor("offv", [128, NCH], dt.bfloat16,
                          kind="ExternalInput").ap()
    iotain = nc.dram_tensor("iotain", [128, SPAN], dt.bfloat16,
                            kind="ExternalInput").ap()
    povirt = nc.dram_tensor("povirt", [128, NW4, CIMG], dt.bfloat16,
                            kind="ExternalOutput").ap()
    BQ = 32                     # chunks per input DMA batch
    OB = 8                      # bank-groups per output DMA batch
    with tile.TileContext(nc) as tc:
        with tc.tile_pool(name="const", bufs=1) as cpool, \
             tc.tile_pool(name="io", bufs=4) as iop, \
             tc.tile_pool(name="g", bufs=4) as gp, \
             tc.tile_pool(name="ps", bufs=4, space="PSUM") as pp:
            iota = cpool.tile([128, SPAN], dt.bfloat16, name="iota")
            nc.sync.dma_start(out=iota[:], in_=iotain)
            offt = cpool.tile([128, NCH], dt.bfloat16, name="offt")
            nc.sync.dma_start(out=offt[:], in_=offv)
            ps = None
            ot = None
            for w in range(W):
                k0 = w * QV
                qr = w % 4
                if k0 % BQ == 0:
                    pbt = iop.tile([128, BQ, CIMG], dt.bfloat16,
                                   tag="pbt", name="pbt")
                    nb = min(BQ, NCH - k0)
                    nc.sync.dma_start(out=pbt[:, 0:nb, :],
                                      in_=pb[:, k0:k0 + nb, :])
                if k0 % 16 == 0:
                    # one-hot rows for 16 chunks in one batched DVE op
                    g16 = gp.tile([128, 16, SPAN], dt.bfloat16, tag="g",
                                  name="g16")
                    ia = bass.AP(iota.tensor, iota.offset,
                                 [iota.ap[0], [0, 16], [1, SPAN]])
                    ob = bass.AP(offt.tensor, offt.offset + k0,
                                 [offt.ap[0], [1, 16], [0, SPAN]])
                    nc.vector.tensor_tensor(out=g16[:], in0=ia, in1=ob,
                                            op=mybir.AluOpType.is_equal)
                if qr == 0:
                    ps = pp.tile([128, CIMG], dt.float32, tag="ps", name="ps")
                for q in range(QV):
                    k = k0 + q
                    nc.tensor.matmul(ps[qr * 32:(qr + 1) * 32, :],
                                     g16[:, k % 16, :], pbt[:, k % BQ, :],
                                     start=(q == 0), stop=(q == QV - 1),
                                     tile_position=(0, qr * 32))
                if qr == 3:
                    b = w // 4
                    if b % OB == 0:
                        ot = iop.tile([128, OB, CIMG], dt.bfloat16,
                                      tag="ot", name="ot")
                    nc.scalar.activation(ot[:, b % OB, :], ps[:],
                                         mybir.ActivationFunctionType.Copy)
                    if b % OB == OB - 1 or b == NW4 - 1:
                        b0 = (b // OB) * OB
                        nc.sync.dma_start(out=povirt[:, b0:b + 1, :],
                                          in_=ot[:, 0:b + 1 - b0, :])
    nc.compile()
    return nc


# ---------------------------------------------------------------- launch C
C_OUT_ROWS = 23              # ds2-out rows per core (8*23 = 184 >= 180)


def build_launch_c():
    nc = bacc.Bacc("TRN2", target_bir_lowering=False, debug=False,
                   num_devices=NCORES)
    NR1 = C_OUT_ROWS + 2                         # ds1-out rows incl halo (25)
    NRP = 2 * NR1 + 1                            # pooled rows needed (51)
    slab = nc.dram_tensor("slab", [CIMG, NRP, 362], dt.bfloat16,
                          kind="ExternalInput").ap()
    m1 = nc.dram_tensor("m1", [128, NR1], dt.bfloat16, kind="ExternalInput").ap()
    wd1 = nc.dram_tensor("wd1", [9, CIMG, CIMG], dt.bfloat16,
                         kind="ExternalInput").ap()
    wd2 = nc.dram_tensor("wd2", [9, CIMG, CIMG], dt.bfloat16,
                         kind="ExternalInput").ap()
    sb1 = nc.dram_tensor("sb1", [CIMG, 2], dt.float32, kind="ExternalInput").ap()
    sb2 = nc.dram_tensor("sb2", [CIMG, 2], dt.float32, kind="ExternalInput").ap()
    yout = nc.dram_tensor("yout", [CIMG, C_OUT_ROWS, 180], dt.float32,
                          kind="ExternalOutput").ap()
    RELU = mybir.ActivationFunctionType.Relu
    with tile.TileContext(nc) as tc:
        with tc.tile_pool(name="const", bufs=1) as cpool, \
             tc.tile_pool(name="work", bufs=2) as wp, \
             tc.tile_pool(name="big", bufs=1) as bp, \
             tc.tile_pool(name="ps", bufs=4, space="PSUM") as pp:
            w1 = cpool.tile([CIMG, 9, CIMG], dt.bfloat16, name="w1")
            nc.sync.dma_start(out=w1[:], in_=wd1.rearrange("t p o -> p t o"))
            sb1t = cpool.tile([CIMG, 2], dt.float32, name="sb1t")
            nc.sync.dma_start(out=sb1t[:], in_=sb1)
            slabt = bp.tile([CIMG, NRP, 362], dt.bfloat16, name="slabt")
            for rr in range(0, NRP, 7):
                nrr = min(7, NRP - rr)
                nc.sync.dma_start(out=slabt[:, rr:rr + nrr, :],
                                  in_=slab[:, rr:rr + nrr, :])
            w2 = cpool.tile([CIMG, 9, CIMG], dt.bfloat16, name="w2")
            nc.sync.dma_start(out=w2[:], in_=wd2.rearrange("t p o -> p t o"))
            sb2t = cpool.tile([CIMG, 2], dt.float32, name="sb2t")
            nc.sync.dma_start(out=sb2t[:], in_=sb2)
            m1t = wp.tile([128, NR1], dt.bfloat16, name="m1t")
            nc.sync.dma_start(out=m1t[:], in_=m1)
            h1 = bp.tile([CIMG, NR1, 182], dt.bfloat16, name="h1")
            nc.vector.memset(h1[:, :, 0:1], 0.0)
            nc.vector.memset(h1[:, :, 181:182], 0.0)
            # ds1: stride-2 3x3; out row t reads slab rows 2t..2t+2 (slab row 0
            # = pooled row 2o0-3, so out row t (global o0-1+t) reads
            # 2(o0-1+t)-1..+1 - (2o0-3) = 2t..2t+2); col c reads 2c..2c+2
            RP = 2
            for t0 in range(0, NR1, RP):
                nr = min(RP, NR1 - t0)
                ps = pp.tile([CIMG, nr, 180], dt.float32, tag="ps1", name="ps")
                gi = 0
                for ky in range(3):
                    for kx in range(3):
                        rhs = bass.AP(slabt.tensor,
                                      slabt.offset + (2 * t0 + ky) * 362 + kx,
                                      [slabt.ap[0], [2 * 362, nr], [2, 180]])
                        nc.tensor.matmul(ps[:], w1[:, ky * 3 + kx, :], rhs,
                                         start=(gi == 0), stop=(gi == 8))
                        gi += 1
                nc.scalar.activation(h1[:, t0:t0 + nr, 1:181], ps[:], RELU,
                                     bias=sb1t[:, 1:2], scale=sb1t[:, 0:1])
            # zero rows outside the global [0,180) output range (edges only)
            mlo = bass.AP(m1t.tensor, m1t.offset,
                          [[m1t.ap[0][0], CIMG], [1, 1], [0, 182]])
            nc.vector.tensor_tensor(out=h1[:, 0:1, :], in0=h1[:, 0:1, :],
                                    in1=mlo, op=mybir.AluOpType.mult)
            mhi = bass.AP(m1t.tensor, m1t.offset + NR1 - 5,
                          [[m1t.ap[0][0], CIMG], [1, 5], [0, 182]])
            nc.vector.tensor_tensor(out=h1[:, NR1 - 5:NR1, :],
                                    in0=h1[:, NR1 - 5:NR1, :],
                                    in1=mhi, op=mybir.AluOpType.mult)
            # ds2: 3x3 pad 1: out row o reads h1 rows o..o+2, col c: c..c+2
            yo = bp.tile([CIMG, C_OUT_ROWS, 180], dt.float32, name="yo")
            for o0 in range(0, C_OUT_ROWS, RP):
                nr = min(RP, C_OUT_ROWS - o0)
                ps = pp.tile([CIMG, nr, 180], dt.float32, tag="ps2", name="ps")
                gi = 0
                for ky in range(3):
                    for kx in range(3):
                        rhs = bass.AP(h1.tensor,
                                      h1.offset + (o0 + ky) * 182 + kx,
                                      [h1.ap[0], [182, nr], [1, 180]])
                        nc.tensor.matmul(ps[:], w2[:, ky * 3 + kx, :], rhs,
                                         start=(gi == 0), stop=(gi == 8))
                        gi += 1
                nc.scalar.activation(yo[:, o0:o0 + nr, :], ps[:], RELU,
                                     bias=sb2t[:, 1:2], scale=sb2t[:, 0:1])
                nc.sync.dma_start(out=yout[:, o0:o0 + nr, :],
                                  in_=yo[:, o0:o0 + nr, :])
    nc.compile()
    return nc


_CACHE = {}


def run_launch_a(inputs):
    if "A" not in _CACHE:
        _CACHE["A"] = build_launch_a()
    nc = _CACHE["A"]
    maps = _prep_a_inputs(inputs)
    res = run_bass_kernel_spmd(nc, maps, list(range(NCORES)))
    NPC0 = (SEGS[0]["nout"] * FW + 127) // 128
    NPC1 = (SEGS[1]["nout"] * FW + 127) // 128
    depth = np.zeros((NPIX, DD), np.float32)
    feat = np.zeros((NPIX, CIMG), np.float32)
    for c in range(NCORES):
        r = res.results[c]
        for s, (cam, h0) in enumerate([SEG_A[c], SEG_B[c]]):
            S = SEGS[s]
            npix = S["nout"] * FW
            base = (cam * FH + h0) * FW
            b0, npc = (0, NPC0) if s == 0 else (NPC0, NPC1)
            darr = r["out_depth"][:, b0:b0 + npc].transpose(1, 0, 2)
            farr = r["out_feat"][:, b0:b0 + npc].transpose(1, 0, 2)
            depth[base:base + npix] = darr.reshape(-1, DD)[:npix]
            feat[base:base + npix] = farr.reshape(-1, CIMG)[:npix].astype(
                np.float32)
    return depth, feat


def _build_schedule(flat, kept):
    """Sort kept points by (core, local voxel); emit fixed-quota virtual
    windows of QV*128 points with vox-span < SPAN. Returns per-core schedule
    dicts + W (max window count, rounded to 8)."""
    pts = np.arange(NPTS)
    rem = pts % (DD * FH * FW)
    d_i = rem // (FH * FW)
    col = (pts // (DD * FH * FW)) * (FH * FW) + rem % (FH * FW)
    vox = flat
    vx = (vox // NX).astype(np.int32)

    keep_idx = np.where(kept)[0]
    cnt = np.bincount(vx[keep_idx], minlength=NX)
    order = np.argsort(-cnt, kind="stable")
    core_of_row = np.zeros(NX, np.int32)
    load = np.zeros(NCORES, np.int64)
    for r in order:
        c = int(np.argmin(load))
        core_of_row[r] = c
        load[c] += cnt[r]

    row_rank = np.zeros(NX, np.int64)
    rows_of = []
    for c in range(NCORES):
        rows = np.where(core_of_row == c)[0]
        rows_of.append(rows)
        row_rank[rows] = np.arange(len(rows))

    schedules = []
    for c in range(NCORES):
        sel = keep_idx[core_of_row[vx[keep_idx]] == c]
        vloc = row_rank[vx[sel]] * NX + (vox[sel] % NX)
        o = np.argsort(vloc, kind="stable")
        sel, vloc = sel[o], vloc[o]
        win = []                      # (start, end, base)
        i, n = 0, len(sel)
        while i < n:
            base = vloc[i]
            j = min(i + QV * 128, n)
            hi = np.searchsorted(vloc, base + SPAN, "left")
            j = min(j, hi)
            win.append((i, j, base))
            i = j
        schedules.append(dict(sel=sel, vloc=vloc, win=win, col=col[sel],
                              d_i=d_i[sel], rows=rows_of[c]))
    W = max(len(s["win"]) for s in schedules)
    W = (W + 7) // 8 * 8
    return schedules, W


def _prep_b_inputs(schedules, W, depth_rows, featflat):
    maps = []
    NCH = W * QV
    iota = np.broadcast_to(np.arange(SPAN, dtype=np.float32)[None, :],
                           (128, SPAN)).astype(bf16)
    for sch in schedules:
        pb = np.zeros((128, NCH, CIMG), bf16)
        offv = np.zeros((128, NCH), bf16)
        col, d_i, vloc = sch["col"], sch["d_i"], sch["vloc"]
        dvals = depth_rows[col, d_i]                  # f32 depth weights
        wfeat = (dvals[:, None] * featflat[col]).astype(bf16)
        for w, (i, j, base) in enumerate(sch["win"]):
            L = j - i
            nch = (L + 127) // 128
            for q in range(nch):
                lo, hi = q * 128, min((q + 1) * 128, L)
                k = w * QV + q
                pb[0:hi - lo, k] = wfeat[i + lo:i + hi]
                offv[0:hi - lo, k] = (vloc[i + lo:i + hi] - base).astype(
                    np.float32)
        maps.append(dict(pb=pb, offv=offv, iotain=iota))
    return maps


def _prep_c_inputs(inputs, pooled_t):
    """pooled_t: [CIMG, 360, 360] f32 -> per-core slabs + masks + weights."""
    NR1 = C_OUT_ROWS + 2
    NRP = 2 * NR1 + 1
    w1 = np.asarray(inputs["ds1_w"], np.float32)
    w2 = np.asarray(inputs["ds2_w"], np.float32)
    wd1 = np.stack([w1[:, :, ky, kx].T for ky in range(3) for kx in range(3)])
    wd2 = np.stack([w2[:, :, ky, kx].T for ky in range(3) for kx in range(3)])
    sb1 = np.stack([np.asarray(inputs["ds1_s"], np.float32),
                    np.asarray(inputs["ds1_t"], np.float32)], 1)
    sb2 = np.stack([np.asarray(inputs["ds2_s"], np.float32),
                    np.asarray(inputs["ds2_t"], np.float32)], 1)
    shared = dict(wd1=wd1.astype(bf16), wd2=wd2.astype(bf16), sb1=sb1, sb2=sb2)
    maps = []
    pt_bf = pooled_t.astype(bf16)
    for c in range(NCORES):
        o0g = C_OUT_ROWS * c
        p0 = 2 * o0g - 3
        slab = np.zeros((CIMG, NRP, 362), bf16)
        lo, hi = max(0, p0), min(NX, p0 + NRP)
        if hi > lo:
            slab[:, lo - p0:hi - p0, 1:361] = pt_bf[:, lo:hi, :]
        t1g = np.arange(NR1) + (o0g - 1)
        m1 = np.broadcast_to(((t1g >= 0) & (t1g < 180))[None, :],
                             (128, NR1)).astype(bf16)
        maps.append(dict(shared, slab=slab, m1=np.ascontiguousarray(m1)))
    return maps


def kernel(**inputs):
    inputs = {k: np.asarray(v) for k, v in inputs.items()}
    flat, kept = _host_geometry(inputs["cam2lidar_rots"],
                                inputs["cam2lidar_trans"], inputs["intrins"],
                                inputs["post_rots"], inputs["post_trans"])
    depth_rows, feat_rows = run_launch_a(inputs)

    schedules, W = _build_schedule(flat, kept)
    key = ("B", W)
    if key not in _CACHE:
        _CACHE[key] = build_launch_b(W)
    bmaps = _prep_b_inputs(schedules, W, depth_rows, feat_rows)
    res_b = run_bass_kernel_spmd(_CACHE[key], bmaps, list(range(NCORES)))

    pooled = np.zeros((NX * NX, CIMG), np.float32)
    for c, sch in enumerate(schedules):
        virt = res_b.results[c]["povirt"].astype(np.float32)  # [128, NW4, C]
        rows_arr = sch["rows"]
        nloc = len(rows_arr) * NX
        for w, (i, j, base) in enumerate(sch["win"]):
            span = min(SPAN, nloc - base)
            lidx = base + np.arange(span)
            ridx = rows_arr[lidx // NX] * NX + (lidx % NX)
            qr = w % 4
            pooled[ridx] += virt[qr * 32:qr * 32 + span, w // 4]
    pooled_t = np.ascontiguousarray(
        pooled.reshape(NX, NX, CIMG).transpose(2, 0, 1))

    if "C" not in _CACHE:
        _CACHE["C"] = build_launch_c()
    cmaps = _prep_c_inputs(inputs, pooled_t)
    res_c = run_bass_kernel_spmd(_CACHE["C"], cmaps, list(range(NCORES)))
    out = np.zeros((1, CIMG, 180, 180), np.float32)
    for c in range(NCORES):
        o0g = C_OUT_ROWS * c
        nr = min(C_OUT_ROWS, 180 - o0g)
        if nr > 0:
            out[0, :, o0g:o0g + nr, :] = res_c.results[c]["yout"][:, 0:nr, :]
    return out

